# revision 34
# baseline (speedup 1.0000x reference)
"""Trainium2 Bass kernel for the 2-hop key-value memory network.

v2 strategy: data-parallel over batch (B=32 -> 4 per core x 8 cores).
Per core:
  - 12 gather streams (b-outer, tau in s/r/o), each a lo/hi int16-split
    compaction of 4096 token gathers from a host-concatenated
    [vocab, 3*m] bf16 table. Exact (16-aligned) static sizes per stream
    (max over cores) instead of per-chunk 128-rounding: ~3.5% padding.
  - Selection matrices generated ON-CHIP per 128-slot group via
    tensor_scalar(is_equal) of an iota row-constant against per-slot
    mem-ids (uploaded, tiny). Word-sum + slot->mem routing via PE
    matmuls accumulating in PSUM, two phases (lo then hi) per mem-chunk
    with an SBUF add between, so only ~2 chunk accumulators live at once.
  - E0T/E1T transposes inline as chunks complete; hop-0 scores, renorm,
    o-sums and u-update inline per batch as its 3 streams finish.
  - log(sum_exp) via exponent-extraction + cubic log2 poly on DVE
    (no Ln activation-table loads; Exp table loaded once at ramp).
The log-softmax renorms reduce to per-row affine transforms:
  renorm_q: p = (s - min(s)) / (sum(s) - 512*min(s) + 512e-8)
  renorm_a: p = (s - lse)   / (512*lse - sum(s)),  lse = logsumexp(s)
"""
import sys

for _p in ("/opt/pypackages", "/opt/trn_rl_repo"):
    if _p not in sys.path:
        sys.path.insert(0, _p)

import numpy as np
import ml_dtypes

import concourse.bass as bass
import concourse.bacc as bacc
import concourse.mybir as mybir
import concourse.tile as tile
from concourse.bass_utils import run_bass_kernel_spmd

BF = ml_dtypes.bfloat16

# problem constants
B, NMEM, NW, QLEN, NCH, CLEN = 32, 512, 8, 32, 8, 8
VOCAB, M = 50000, 256
EMB = 3 * M           # 768
NCORES = 8
BL = B // NCORES      # 4 batch per core
SPLIT = 32768         # int16 index split point
PIECE = 512           # slots per dma_gather piece
NSTREAM = 3 * BL      # 12 streams per core, s = 3*b + tau

# log2 cubic poly on m in [1,2): log2(m) ~ C3*m^3 + C2*m^2 + C1*m + C0
_LOGC = None

_cache = {}
TRACE = False
DEBUG = False
LAST_RESULTS = None


def _log2_poly():
    global _LOGC
    if _LOGC is None:
        m = np.linspace(1.0, 2.0, 4097)
        _LOGC = np.polyfit(m, np.log2(m), 3)   # [C3, C2, C1, C0]
    return _LOGC


def _align16(n):
    return -(-int(n) // 16) * 16


def _wrap_idx16(stream):
    """stream (len multiple of 16) -> [128, len/16] int16 wrapped layout:
    index i lives at [i % 16, i // 16], replicated across the 8 groups of 16
    partitions."""
    n = len(stream)
    c = n // 16
    arr = np.zeros((16, c), dtype=np.uint16)
    arr[:, :] = stream.astype(np.uint16).reshape(c, 16).T
    return np.tile(arr, (8, 1)).view(np.int16)


def _stream_tokens(subjects, relations, objects, b, tau):
    toks = (subjects, relations, objects)[tau]
    return toks[b].reshape(-1)     # [4096] mem-major


def _build_structure(subjects, relations, objects):
    """Static (all-core-union) structure: per-stream lo/hi sizes and
    group -> sorted chunk list."""
    LO, HI, gmap = [], [], []
    for s in range(NSTREAM):
        b, tau = divmod(s, 3)
        max_lo = max_hi = 0
        per_core = []
        for core in range(NCORES):
            t = _stream_tokens(subjects[core * BL:(core + 1) * BL],
                               relations[core * BL:(core + 1) * BL],
                               objects[core * BL:(core + 1) * BL], b, tau)
            lo_m = t < SPLIT
            mems = np.arange(4096) // NW
            lo_mm, hi_mm = mems[lo_m], mems[~lo_m]
            per_core.append((lo_mm, hi_mm))
            max_lo = max(max_lo, len(lo_mm))
            max_hi = max(max_hi, len(hi_mm))
        lo_c, hi_c = _align16(max_lo), _align16(max_hi)
        nglo = -(-lo_c // 128)
        nghi = -(-hi_c // 128)
        gm = [set() for _ in range(nglo + nghi)]
        for lo_mm, hi_mm in per_core:
            for g in range(nglo):
                seg = lo_mm[128 * g:128 * (g + 1)]
                if len(seg):
                    gm[g].update(np.unique(seg // 128).tolist())
            for g in range(nghi):
                seg = hi_mm[128 * g:128 * (g + 1)]
                if len(seg):
                    gm[nglo + g].update(np.unique(seg // 128).tolist())
        LO.append(lo_c)
        HI.append(hi_c)
        gmap.append(tuple(tuple(sorted(x)) for x in gm))
    return tuple(LO), tuple(HI), tuple(gmap)


def _build_core_data(subjects, relations, objects, LO, HI):
    """Per-core idx16 (padded to CMAX cols) and memid (padded to NGMAX)."""
    cmax = max((LO[s] + HI[s]) // 16 for s in range(NSTREAM))
    ngmax = max(-(-LO[s] // 128) - (-HI[s] // 128) for s in range(NSTREAM))
    idx16 = np.zeros((NSTREAM, 128, cmax), dtype=np.int16)
    memid = np.full((NSTREAM, 128, ngmax), 9999.0, dtype=np.float32)
    for s in range(NSTREAM):
        b, tau = divmod(s, 3)
        t = _stream_tokens(subjects, relations, objects, b, tau)
        lo_m = t < SPLIT
        mems = np.arange(4096) // NW
        lo_v, lo_mm = t[lo_m], mems[lo_m]
        hi_v, hi_mm = t[~lo_m] - SPLIT, mems[~lo_m]
        lo_c, hi_c = LO[s], HI[s]
        if len(lo_v) > lo_c or len(hi_v) > hi_c:
            raise OverflowError("stream overflow")
        stream = np.zeros(lo_c + hi_c, dtype=np.int64)
        stream[:len(lo_v)] = lo_v
        stream[lo_c:lo_c + len(hi_v)] = hi_v
        idx16[s, :, :(lo_c + hi_c) // 16] = _wrap_idx16(stream)
        nglo = -(-lo_c // 128)
        for g in range(nglo):
            seg = lo_mm[128 * g:128 * (g + 1)]
            memid[s, :len(seg), g] = seg.astype(np.float32)
        for g in range(-(-hi_c // 128)):
            seg = hi_mm[128 * g:128 * (g + 1)]
            memid[s, :len(seg), nglo + g] = seg.astype(np.float32)
    return idx16, memid, cmax, ngmax


def _pieces(size):
    """Split a section of `size` slots into gather pieces <= PIECE."""
    out = []
    off = 0
    while off < size:
        n = min(PIECE, size - off)
        out.append((off, n))
        off += n
    return out


def _runs(gmap_s, nglo, ngroups):
    """Per (chunk, phase) -> (first_group, last_group) over the union map.
    phase 0 = lo (groups < nglo), phase 1 = hi."""
    runs = {}
    for g, cs in enumerate(gmap_s):
        ph = 0 if g < nglo else 1
        for c in cs:
            k = (c, ph)
            if k not in runs:
                runs[k] = [g, g]
            runs[k][0] = min(runs[k][0], g)
            runs[k][1] = max(runs[k][1], g)
    return runs


def _build_program(LO, HI, gmap, cmax, ngmax):
    key = (LO, HI, gmap, DEBUG)
    if key in _cache:
        return _cache[key]

    f32 = mybir.dt.float32
    bf16 = mybir.dt.bfloat16
    C3, C2, C1, C0 = [float(c) for c in _log2_poly()]
    LN2 = float(np.log(2.0))

    nc = bacc.Bacc("TRN2", target_bir_lowering=False, debug=False)
    a_cat = nc.dram_tensor("a_cat", [VOCAB, EMB], bf16, kind="ExternalInput")
    b_tab = nc.dram_tensor("b_tab", [VOCAB, EMB], bf16, kind="ExternalInput")
    ut_d = nc.dram_tensor("ut", [EMB, EMB], bf16, kind="ExternalInput")
    vt_d = nc.dram_tensor("vt", [EMB, EMB], bf16, kind="ExternalInput")
    w_d = nc.dram_tensor("w", [EMB, EMB], bf16, kind="ExternalInput")
    idx16_d = nc.dram_tensor("idx16", [NSTREAM, 128, cmax], mybir.dt.int16,
                             kind="ExternalInput")
    memid_d = nc.dram_tensor("memid", [NSTREAM, 128, ngmax], f32,
                             kind="ExternalInput")
    iota_d = nc.dram_tensor("iota", [128, NMEM], mybir.dt.int16,
                           kind="ExternalInput")
    ones3_d = nc.dram_tensor("ones3", [3, 128, 128], bf16, kind="ExternalInput")
    maskq_d = nc.dram_tensor("maskq", [128, 1], f32, kind="ExternalInput")
    identf_d = nc.dram_tensor("identf", [128, 128], f32, kind="ExternalInput")
    identb_d = nc.dram_tensor("identb", [128, 128], bf16, kind="ExternalInput")
    idxua_d = nc.dram_tensor("idxua", [128, 3], mybir.dt.int32,
                             kind="ExternalInput")
    out_d = nc.dram_tensor("pred", [BL, NCH], f32, kind="ExternalOutput")
    if DEBUG:
        e_dbg = nc.dram_tensor("e_dbg", [NSTREAM, 128, 4, EMB], bf16,
                               kind="ExternalOutput")
        e0t_dbg = nc.dram_tensor("e0t_dbg", [BL, 128, 6, NMEM], bf16,
                                 kind="ExternalOutput")
        e1t_dbg = nc.dram_tensor("e1t_dbg", [BL, 128, 6, NMEM], bf16,
                                 kind="ExternalOutput")
        x_dbg = nc.dram_tensor("x_dbg", [128, EMB], f32,
                               kind="ExternalOutput")
        p_dbg = nc.dram_tensor("p_dbg", [128, NMEM], bf16,
                               kind="ExternalOutput")

    with tile.TileContext(nc) as tc:
        with (
            tc.tile_pool(name="const", bufs=1) as constp,
            tc.tile_pool(name="state", bufs=1) as statep,
            tc.tile_pool(name="selp1", bufs=16) as selp1,
            tc.tile_pool(name="selp2", bufs=4) as selp2,
            tc.tile_pool(name="wsp", bufs=2, space="PSUM") as wsp,
            tc.tile_pool(name="tpps", bufs=1, space="PSUM") as tpps,
            tc.tile_pool(name="yps", bufs=1, space="PSUM") as ypsp,
            tc.tile_pool(name="ops", bufs=1, space="PSUM") as opsp,
            tc.tile_pool(name="hps", bufs=1, space="PSUM") as hpsp,
        ):
            # ---- small consts first (gathers depend on idx/memid) ----
            idx_sb = [constp.tile([128, cmax], mybir.dt.int16, tag=f"idx{i}",
                                  name=f"idx{i}") for i in range(2)]
            mem_sb = [constp.tile([128, ngmax], f32, tag=f"mem{s}",
                                  name=f"mem{s}") for s in range(NSTREAM)]
            nc.sync.dma_start(out=idx_sb[0][:], in_=idx16_d[0])
            nc.sync.dma_start(out=mem_sb[0][:], in_=memid_d[0])
            iota = constp.tile([128, NMEM], mybir.dt.int16, tag="iota")
            nc.sync.dma_start(out=iota[:], in_=iota_d[:])
            identb = constp.tile([128, 128], bf16, tag="identb")
            nc.sync.dma_start(out=identb[:], in_=identb_d[:])

            # persistent state
            X = statep.tile([128, EMB], f32, tag="X")
            E = [statep.tile([128, 4, EMB], bf16, tag=f"E{s}", name=f"E{s}")
                 for s in range(NSTREAM)]
            E0T = [statep.tile([128, 6, NMEM], bf16, tag=f"E0T{b}",
                               name=f"E0T{b}") for b in range(BL)]
            E1T = [statep.tile([128, 6, NMEM], bf16, tag=f"E1T{b}",
                               name=f"E1T{b}") for b in range(BL)]
            G = [statep.tile([128, 4, EMB], bf16, tag=f"G{i}", name=f"G{i}")
                 for i in range(6)]
            S = hpsp.tile([128, NMEM], f32, tag="S")
            P = statep.tile([128, NMEM], bf16, tag="P")
            PT = statep.tile([128, 4, 128], bf16, tag="PT")
            o_sb = statep.tile([128, EMB], bf16, tag="o_sb")
            # renorm scalars [128,1]
            sc = {n: statep.tile([128, 1], f32, tag=n, name=n)
                  for n in ("mx", "mn", "sm", "negmx", "se", "lse", "ef",
                            "mf", "t1", "Av", "bq", "ba", "Bv", "invb")}

            # memset gather buffers once (NaN-safety for stale group tails)
            for i in range(6):
                nc.any.memset(G[i][:], 0.0)
            nc.vector.memset(S[:], 0.0)
            nc.any.memset(P[:], 0.0)

            # ---- remaining consts ----
            nc.sync.dma_start(out=idx_sb[1][:], in_=idx16_d[1])
            for s in range(1, NSTREAM):
                nc.sync.dma_start(out=mem_sb[s][:], in_=memid_d[s])
            identf = constp.tile([128, 128], f32, tag="identf")
            nc.sync.dma_start(out=identf[:], in_=identf_d[:])
            maskq = constp.tile([128, 1], f32, tag="maskq")
            nc.sync.dma_start(out=maskq[:], in_=maskq_d[:])
            ones3 = [constp.tile([128, 128], bf16, tag=f"ones{i}",
                                 name=f"ones{i}") for i in range(3)]
            for i in range(3):
                nc.sync.dma_start(out=ones3[i][:], in_=ones3_d[i])
            idxua = constp.tile([128, 3], mybir.dt.int32, tag="idxua")
            nc.sync.dma_start(out=idxua[:], in_=idxua_d[:])
            ut_sb = constp.tile([128, 6, EMB], bf16, tag="ut")
            vt_sb = constp.tile([128, 6, EMB], bf16, tag="vt")
            w_sb = constp.tile([128, 6, EMB], bf16, tag="w")
            for t_sb, t_d in ((ut_sb, ut_d), (vt_sb, vt_d), (w_sb, w_d)):
                nc.sync.dma_start(
                    out=t_sb[:],
                    in_=t_d[:].rearrange("(j p) d -> p j d", p=128))

            # ---- init: u and a from B_table -> X; preload Exp table ----
            # stage the 3 B-table gathers in G[0] (free until stream 0)
            for i in range(3):
                nc.gpsimd.indirect_dma_start(
                    out=G[0][:, i, :], out_offset=None, in_=b_tab[:],
                    in_offset=bass.IndirectOffsetOnAxis(
                        ap=idxua[:, i:i + 1], axis=0))
            ps0 = wsp.tile([128, EMB], f32, tag="wsp", name="ps0")
            for i in range(3):
                nc.tensor.matmul(out=ps0[:, 0:512], lhsT=ones3[i][:],
                                 rhs=G[0][:, i, 0:512],
                                 start=(i == 0), stop=(i == 2))
                nc.tensor.matmul(out=ps0[:, 512:768], lhsT=ones3[i][:],
                                 rhs=G[0][:, i, 512:768],
                                 start=(i == 0), stop=(i == 2))
            nc.vector.tensor_copy(out=X[:, 0:512], in_=ps0[:, 0:512])
            nc.vector.tensor_copy(out=X[:, 512:768], in_=ps0[:, 512:768])
            # warm the Exp activation table (only table ever used)
            nc.scalar.activation(out=sc["ef"][:], in_=maskq[:],
                                 func=mybir.ActivationFunctionType.Exp)

            # ---- helpers ----
            def xt_extract(src_sb, ident, xq, xa, dt):
                """Transpose [128, 768] src into per-b q/a column tiles."""
                for j in range(6):
                    tp = tpps.tile([128, 128], dt, tag="tp")
                    nc.tensor.transpose(
                        out=tp[:], in_=src_sb[:, 128 * j:128 * j + 128],
                        identity=ident[:])
                    v = tp[:].rearrange("p (b n) -> p b n", b=4)
                    nc.vector.tensor_copy(out=xq[:, j, :], in_=v[:, :, 0])
                    nc.vector.tensor_copy(out=xa[:, j, :], in_=v[:, :, 1:9])

            def y_project(xq, xa, ysb):
                for i in range(6):
                    y36 = ypsp.tile([128, 36], f32, tag="y36", name="y36")
                    for j in range(6):
                        nc.tensor.matmul(
                            out=y36[:, 0:4],
                            lhsT=ut_sb[:, j, 128 * i:128 * i + 128],
                            rhs=xq[:, j, :], start=(j == 0), stop=(j == 5))
                    for j in range(6):
                        nc.tensor.matmul(
                            out=y36[:, 4:36],
                            lhsT=vt_sb[:, j, 128 * i:128 * i + 128],
                            rhs=xa[:, j, :], start=(j == 0), stop=(j == 5))
                    yv = ysb[:, i, :].rearrange("p (b n) -> p b n", b=4)
                    nc.vector.tensor_copy(out=yv[:, :, 0], in_=y36[:, 0:4])
                    nc.vector.tensor_copy(out=yv[:, :, 1:9], in_=y36[:, 4:36])

            def renorm(rows, n):
                """Renorm S[rows:rows+n] -> P[rows:rows+n] (bf16)."""
                rs = slice(rows, rows + n)
                nc.vector.tensor_reduce(out=sc["mx"][rs], in_=S[rs, :],
                                        axis=mybir.AxisListType.X,
                                        op=mybir.AluOpType.max)
                nc.vector.tensor_reduce(out=sc["mn"][rs], in_=S[rs, :],
                                        axis=mybir.AxisListType.X,
                                        op=mybir.AluOpType.min)
                nc.vector.tensor_reduce(out=sc["sm"][rs], in_=S[rs, :],
                                        axis=mybir.AxisListType.X,
                                        op=mybir.AluOpType.add)
                nc.vector.tensor_scalar(out=sc["negmx"][rs], in0=sc["mx"][rs],
                                        scalar1=-1.0, scalar2=None,
                                        op0=mybir.AluOpType.mult)
                nc.scalar.activation(
                    out=P[rs, :], in_=S[rs, :],
                    func=mybir.ActivationFunctionType.Exp,
                    bias=sc["negmx"][rs], scale=1.0, accum_out=sc["se"][rs])
                # lse = mx + ln2 * log2(se); log2 via exponent + cubic poly
                se_i = sc["se"][rs].bitcast(mybir.dt.int32)
                ef_i = sc["ef"][rs].bitcast(mybir.dt.int32)
                nc.vector.tensor_scalar(out=ef_i, in0=se_i,
                                        scalar1=23, scalar2=None,
                                        op0=mybir.AluOpType.logical_shift_right)
                nc.vector.tensor_copy(out=sc["ef"][rs], in_=ef_i)
                mf_i = sc["mf"][rs].bitcast(mybir.dt.int32)
                nc.vector.tensor_scalar(out=mf_i, in0=se_i,
                                        scalar1=0x007FFFFF,
                                        scalar2=0x3F800000,
                                        op0=mybir.AluOpType.bitwise_and,
                                        op1=mybir.AluOpType.bitwise_or)
                m = sc["mf"][rs]
                t1 = sc["t1"][rs]
                nc.vector.tensor_scalar(out=t1, in0=m, scalar1=C3, scalar2=C2,
                                        op0=mybir.AluOpType.mult,
                                        op1=mybir.AluOpType.add)
                nc.vector.tensor_tensor(out=t1, in0=t1, in1=m,
                                        op=mybir.AluOpType.mult)
                nc.vector.tensor_scalar(out=t1, in0=t1, scalar1=C1,
                                        scalar2=None,
                                        op0=mybir.AluOpType.add)
                nc.vector.tensor_tensor(out=t1, in0=t1, in1=m,
                                        op=mybir.AluOpType.mult)
                nc.vector.tensor_scalar(out=t1, in0=t1, scalar1=C0 - 127.0,
                                        scalar2=None,
                                        op0=mybir.AluOpType.add)
                nc.vector.tensor_tensor(out=t1, in0=t1, in1=sc["ef"][rs],
                                        op=mybir.AluOpType.add)
                nc.vector.tensor_scalar(out=sc["lse"][rs], in0=t1,
                                        scalar1=LN2, scalar2=sc["mx"][rs],
                                        op0=mybir.AluOpType.mult,
                                        op1=mybir.AluOpType.add)
                # A = lse + maskq*(mn - lse)
                nc.vector.tensor_tensor(out=t1, in0=sc["mn"][rs],
                                        in1=sc["lse"][rs],
                                        op=mybir.AluOpType.subtract)
                nc.vector.tensor_tensor(out=t1, in0=t1, in1=maskq[rs],
                                        op=mybir.AluOpType.mult)
                nc.vector.tensor_tensor(out=sc["Av"][rs], in0=sc["lse"][rs],
                                        in1=t1, op=mybir.AluOpType.add)
                # Bq = sm - 512*mn + 512e-8 ; Ba = 512*lse - sm
                nc.vector.tensor_scalar(out=sc["bq"][rs], in0=sc["mn"][rs],
                                        scalar1=-512.0, scalar2=512e-8,
                                        op0=mybir.AluOpType.mult,
                                        op1=mybir.AluOpType.add)
                nc.vector.tensor_tensor(out=sc["bq"][rs], in0=sc["bq"][rs],
                                        in1=sc["sm"][rs],
                                        op=mybir.AluOpType.add)
                nc.vector.tensor_scalar(out=sc["ba"][rs], in0=sc["lse"][rs],
                                        scalar1=512.0, scalar2=None,
                                        op0=mybir.AluOpType.mult)
                nc.vector.tensor_tensor(out=sc["ba"][rs], in0=sc["ba"][rs],
                                        in1=sc["sm"][rs],
                                        op=mybir.AluOpType.subtract)
                nc.vector.tensor_tensor(out=sc["Bv"][rs], in0=sc["bq"][rs],
                                        in1=sc["ba"][rs],
                                        op=mybir.AluOpType.subtract)
                nc.vector.tensor_tensor(out=sc["Bv"][rs], in0=sc["Bv"][rs],
                                        in1=maskq[rs],
                                        op=mybir.AluOpType.mult)
                nc.vector.tensor_tensor(out=sc["Bv"][rs], in0=sc["Bv"][rs],
                                        in1=sc["ba"][rs],
                                        op=mybir.AluOpType.add)
                nc.vector.reciprocal(out=sc["invb"][rs], in_=sc["Bv"][rs])
                nc.vector.tensor_scalar(out=P[rs, :], in0=S[rs, :],
                                        scalar1=sc["Av"][rs],
                                        scalar2=sc["invb"][rs],
                                        op0=mybir.AluOpType.subtract,
                                        op1=mybir.AluOpType.mult)

            # hop-0 Y from initial X (runs during early gathers)
            xtq0 = statep.tile([128, 6, 4], bf16, tag="xtq0")
            xta0 = statep.tile([128, 6, 32], bf16, tag="xta0")
            ysb0 = statep.tile([128, 6, 36], bf16, tag="ysb0")
            xt_extract(X, identf, xtq0, xta0, f32)
            y_project(xtq0, xta0, ysb0)

            # ---- gather streams + word-sum + inline hop 0 ----
            piece_ctr = 0
            pending = []

            def act_copy(out, in_):
                nc.scalar.activation(
                    out=out, in_=in_,
                    func=mybir.ActivationFunctionType.Copy)
            for s in range(NSTREAM):
                b, tau = divmod(s, 3)
                lo_c, hi_c = LO[s], HI[s]
                nglo = -(-lo_c // 128)
                ng = nglo + (-(-hi_c // 128))
                runs = _runs(gmap[s], nglo, ng)
                # chunk psum tiles keyed (c, phase)
                pst = {}
                done_groups = {}

                if s + 1 < NSTREAM:
                    nc.sync.dma_start(out=idx_sb[(s + 1) % 2][:],
                                      in_=idx16_d[s + 1])
                sections = [(0, lo_c, 0, a_cat[:, :]),
                            (nglo, hi_c, lo_c, a_cat[SPLIT:, :])]
                for gbase, size, stream_off, src in sections:
                    for off, n in _pieces(size):
                        gt = G[piece_ctr % 6]
                        piece_ctr += 1
                        npg = -(-n // 128)
                        nc.gpsimd.dma_gather(
                            gt[:, 0:npg, :], src,
                            idx_sb[s % 2][:, (stream_off + off) // 16:
                                          (stream_off + off + n) // 16],
                            n, n, EMB)
                        if pending:
                            pending.pop(0)()
                        g0 = gbase + off // 128
                        sels = {}
                        for gl in range(npg):
                            g = g0 + gl
                            cs = gmap[s][g]
                            if not cs:
                                continue
                            c0, c1 = cs[0], cs[-1]
                            w = 128 * (c1 - c0 + 1)
                            if w > 128:
                                sel = selp2.tile([128, 256], bf16, tag="sl2",
                                                 name="sl2")
                            else:
                                sel = selp1.tile([128, 128], bf16, tag="sl1",
                                                 name="sl1")
                            nc.vector.tensor_scalar(
                                out=sel[:, 0:w],
                                in0=iota[:, 128 * c0:128 * c0 + w],
                                scalar1=mem_sb[s][:, g:g + 1], scalar2=None,
                                op0=mybir.AluOpType.is_equal)
                            sels[gl] = (sel, c0)
                        for gl in range(npg):
                            g = g0 + gl
                            cs = gmap[s][g]
                            if not cs:
                                continue
                            sel, c0 = sels[gl]
                            ph = 0 if g < nglo else 1
                            for c in cs:
                                k = (c, ph)
                                if k not in pst:
                                    pst[k] = wsp.tile([128, EMB], f32,
                                                      tag="wsp", name="pw")
                                pw = pst[k]
                                pa, pb = pw[:, 0:512], pw[:, 512:768]
                                lhs = sel[:, 128 * (c - c0):
                                          128 * (c - c0) + 128]
                                first = runs[k][0] == g
                                last = runs[k][1] == g
                                nc.tensor.matmul(
                                    out=pa, lhsT=lhs, rhs=gt[:, gl, 0:512],
                                    start=first, stop=last)
                                nc.tensor.matmul(
                                    out=pb, lhsT=lhs,
                                    rhs=gt[:, gl, 512:768],
                                    start=first, stop=last)
                                if last:
                                    done_groups[k] = True
                                    pst.pop(k)
                                    if ph == 0:
                                        act_copy(E[s][:, c, 0:512], pa)
                                        act_copy(E[s][:, c, 512:768], pb)
                                    else:
                                        nc.vector.tensor_tensor(
                                            out=E[s][:, c, 0:512],
                                            in0=E[s][:, c, 0:512], in1=pa,
                                            op=mybir.AluOpType.add)
                                        nc.vector.tensor_tensor(
                                            out=E[s][:, c, 512:768],
                                            in0=E[s][:, c, 512:768],
                                            in1=pb,
                                            op=mybir.AluOpType.add)
                                        # inline transposes for E0T / E1T
                                        for q in range(2):
                                            tp = tpps.tile([128, 128], bf16,
                                                           tag="tp")
                                            nc.tensor.transpose(
                                                out=tp[:],
                                                in_=E[s][:, c, 128 * q:
                                                         128 * q + 128],
                                                identity=identb[:])
                                            act_copy(
                                                E0T[b][:, 2 * tau + q,
                                                       128 * c:
                                                       128 * c + 128],
                                                tp[:])
                                        for q in range(2):
                                            tp = tpps.tile([128, 128], bf16,
                                                           tag="tp")
                                            nc.tensor.transpose(
                                                out=tp[:],
                                                in_=E[s][:, c, 256 + 128 * q:
                                                         256 + 128 * q + 128],
                                                identity=identb[:])
                                            act_copy(
                                                E1T[b][:, 2 * tau + q,
                                                       128 * c:
                                                       128 * c + 128],
                                                tp[:])

                # ---- inline hop 0 for batch b once its 3 streams done ----
                if tau == 2:
                    def hop0_scores(b=b):
                        r0 = 32 * b
                        for j in range(6):
                            nc.tensor.matmul(
                                out=S[r0:r0 + 9, :],
                                lhsT=ysb0[:, j, 9 * b:9 * b + 9],
                                rhs=E0T[b][:, j, :], start=(j == 0),
                                stop=(j == 5), tile_position=(0, r0))

                    def hop0_renorm(b=b):
                        renorm(32 * b, 9)

                    def hop0_pt(b=b):
                        r0 = 32 * b
                        for k in range(4):
                            tp = tpps.tile([128, 128], bf16, tag="tp")
                            nc.tensor.transpose(
                                out=tp[:],
                                in_=P[:, 128 * k:128 * k + 128],
                                identity=identb[:])
                            nc.vector.tensor_copy(out=PT[:, k, r0:r0 + 9],
                                                  in_=tp[:, r0:r0 + 9])

                    def hop0_o(b=b, t2=0):
                        r0 = 32 * b
                        if t2 == 0:
                            odst = S[r0:r0 + 9, 0:256]
                        elif t2 == 1:
                            odst = S[r0:r0 + 9, 256:512]
                        else:
                            ot = opsp.tile([128, 256], f32, tag="ot",
                                           name="ot")
                            hop0_o.ot = ot
                            odst = ot[r0:r0 + 9, :]
                        for k in range(4):
                            nc.tensor.matmul(
                                out=odst,
                                lhsT=PT[:, k, r0:r0 + 9],
                                rhs=E[3 * b + t2][:, k, 256:512],
                                start=(k == 0), stop=(k == 3),
                                tile_position=(0, r0))
                        nc.vector.tensor_tensor(
                            out=X[r0:r0 + 9, 256 * t2:256 * t2 + 256],
                            in0=X[r0:r0 + 9, 256 * t2:256 * t2 + 256],
                            in1=odst,
                            op=mybir.AluOpType.add)

                    if b < BL - 1:
                        pending.extend([
                            hop0_scores, hop0_renorm, hop0_pt,
                            lambda b=b: hop0_o(b, 0),
                            lambda b=b: hop0_o(b, 1),
                            lambda b=b: hop0_o(b, 2)])
                    else:
                        hop0_scores()
                        hop0_renorm()
                        hop0_pt()
                        for t2 in range(3):
                            hop0_o(b, t2)

            while pending:
                pending.pop(0)()

            if DEBUG:
                for s in range(NSTREAM):
                    nc.sync.dma_start(out=e_dbg[s], in_=E[s][:])
                for b in range(BL):
                    nc.sync.dma_start(out=e0t_dbg[b], in_=E0T[b][:])
                    nc.sync.dma_start(out=e1t_dbg[b], in_=E1T[b][:])
                nc.sync.dma_start(out=x_dbg[:], in_=X[:])
                nc.sync.dma_start(out=p_dbg[:], in_=P[:])

            # ---------------- hop 1 + final ----------------
            xtq1 = statep.tile([128, 6, 4], bf16, tag="xtq1")
            xta1 = statep.tile([128, 6, 32], bf16, tag="xta1")
            ysb1 = statep.tile([128, 6, 36], bf16, tag="ysb1")
            xt_extract(X, identf, xtq1, xta1, f32)
            y_project(xtq1, xta1, ysb1)
            for b in range(BL):
                r0 = 32 * b
                for j in range(6):
                    nc.tensor.matmul(
                        out=S[r0:r0 + 9, :], lhsT=ysb1[:, j, 9 * b:9 * b + 9],
                        rhs=E1T[b][:, j, :], start=(j == 0), stop=(j == 5),
                        tile_position=(0, r0))
            renorm(0, 128)
            for k in range(4):
                tp = tpps.tile([128, 128], bf16, tag="tp")
                nc.tensor.transpose(out=tp[:], in_=P[:, 128 * k:128 * k + 128],
                                    identity=identb[:])
                nc.vector.tensor_copy(out=PT[:, k, :], in_=tp[:])
            ot1 = opsp.tile([128, 256], f32, tag="ot", name="ot1")
            nc.vector.memset(ot1[:], 0.0)
            for b in range(BL):
                r0 = 32 * b
                odst = [S[r0:r0 + 9, 0:256], S[r0:r0 + 9, 256:512],
                        ot1[r0:r0 + 9, :]]
                for t2 in range(3):
                    for k in range(4):
                        nc.tensor.matmul(
                            out=odst[t2],
                            lhsT=PT[:, k, r0:r0 + 9],
                            rhs=E[3 * b + t2][:, k, 512:768],
                            start=(k == 0), stop=(k == 3),
                            tile_position=(0, r0))
            nc.vector.tensor_copy(out=o_sb[:, 0:256], in_=S[:, 0:256])
            nc.vector.tensor_copy(out=o_sb[:, 256:512], in_=S[:, 256:512])
            nc.vector.tensor_copy(out=o_sb[:, 512:768], in_=ot1[:])

            # final bilinear form: pred[b,c] = o_q[b] . (W @ o_a[c,b])
            otq = statep.tile([128, 6, 4], bf16, tag="otq")
            ota = statep.tile([128, 6, 32], bf16, tag="ota")
            xt_extract(o_sb, identb, otq, ota, bf16)
            wq = statep.tile([128, 6, 4], bf16, tag="wq")
            for i in range(6):
                wqp = ypsp.tile([128, 36], f32, tag="y36", name="wqp")
                for j in range(6):
                    nc.tensor.matmul(
                        out=wqp[:, 0:4],
                        lhsT=w_sb[:, j, 128 * i:128 * i + 128],
                        rhs=otq[:, j, :], start=(j == 0), stop=(j == 5))
                nc.vector.tensor_copy(out=wq[:, i, :], in_=wqp[:, 0:4])
            predp36 = ypsp.tile([128, 36], f32, tag="y36", name="predp36")
            predp = predp36[:, 0:NCH]
            pred_sb = statep.tile([128, NCH], f32, tag="pred_sb")
            for b in range(BL):
                for i in range(6):
                    nc.tensor.matmul(
                        out=predp36[32 * b:32 * b + 1, 0:NCH],
                        lhsT=wq[:, i, b:b + 1],
                        rhs=ota[:, i, 8 * b:8 * b + 8],
                        start=(i == 0), stop=(i == 5),
                        tile_position=(0, 32 * b))
                nc.vector.tensor_copy(out=pred_sb[32 * b:32 * b + 1, :],
                                      in_=predp36[32 * b:32 * b + 1, 0:NCH])
                nc.sync.dma_start(out=out_d[b:b + 1, :],
                                  in_=pred_sb[32 * b:32 * b + 1, :])

    nc.compile()
    _cache[key] = nc
    return nc


def prepare(subjects, relations, objects, ques, answerChoices,
            A_tables, B_table, U, V, W):
    subjects = np.asarray(subjects).astype(np.int64)
    relations = np.asarray(relations).astype(np.int64)
    objects = np.asarray(objects).astype(np.int64)
    ques = np.asarray(ques).astype(np.int64)
    answerChoices = np.asarray(answerChoices).astype(np.int64)
    A_tables = np.asarray(A_tables, dtype=np.float32)
    B_table = np.asarray(B_table, dtype=np.float32)

    # shared (core-independent) device data
    a_cat = np.concatenate([A_tables[0], A_tables[1], A_tables[2]],
                           axis=1).astype(BF)
    b_bf = B_table.astype(BF)
    ut = np.ascontiguousarray(np.asarray(U, dtype=np.float32).T).astype(BF)
    vt = np.ascontiguousarray(np.asarray(V, dtype=np.float32).T).astype(BF)
    w_bf = np.ascontiguousarray(np.asarray(W, dtype=np.float32)).astype(BF)
    identf = np.eye(128, dtype=np.float32)
    identb = np.eye(128, dtype=BF)
    maskq = np.zeros((128, 1), dtype=np.float32)
    maskq[0::32] = 1.0
    iota = np.tile(np.arange(NMEM, dtype=np.int16), (128, 1))
    # init placement matrices (state row = 32*b + 0 for u, +1+c for choices)
    ones3 = np.zeros((3, 128, 128), dtype=BF)
    p = np.arange(128)
    ones3[0, p, 32 * (p // 32)] = 1.0                        # u rows
    ones3[1, p, 32 * (p // 64) + 1 + (p // 8) % 8] = 1.0     # a, b in {0,1}
    ones3[2, p, 32 * (2 + p // 64) + 1 + (p // 8) % 8] = 1.0  # a, b in {2,3}

    LO, HI, gmap = _build_structure(subjects, relations, objects)
    cmax = max((LO[s] + HI[s]) // 16 for s in range(NSTREAM))
    ngmax = max(-(-LO[s] // 128) - (-HI[s] // 128) for s in range(NSTREAM))
    nc = _build_program(LO, HI, gmap, cmax, ngmax)

    in_maps = []
    for core in range(NCORES):
        sl = slice(core * BL, (core + 1) * BL)
        idx16, memid, _, _ = _build_core_data(
            subjects[sl], relations[sl], objects[sl], LO, HI)
        idxua = np.zeros((128, 3), dtype=np.int32)
        idxua[:, 0] = ques[sl][p // 32, p % 32]
        idxua[:, 1] = answerChoices[sl][p // 64, (p // 8) % 8, p % 8]
        idxua[:, 2] = answerChoices[sl][2 + p // 64, (p // 8) % 8, p % 8]
        in_maps.append(dict(
            a_cat=a_cat, b_tab=b_bf, ut=ut, vt=vt, w=w_bf,
            idx16=idx16, memid=memid, iota=iota, ones3=ones3, maskq=maskq,
            identf=identf, identb=identb, idxua=idxua))

    return nc, in_maps


def kernel(**inputs):
    nc, in_maps = prepare(**inputs)
    res = run_bass_kernel_spmd(nc, in_maps, list(range(NCORES)), trace=TRACE)
    global LAST_RESULTS
    LAST_RESULTS = res
    return np.concatenate([res.results[c]["pred"] for c in range(NCORES)],
                          axis=0).astype(np.float32)


# revision 35
# speedup vs baseline: 1.0027x; 1.0027x over previous
"""Trainium2 Bass kernel for the 2-hop key-value memory network.

v2 strategy: data-parallel over batch (B=32 -> 4 per core x 8 cores).
Per core:
  - 12 gather streams (b-outer, tau in s/r/o), each a lo/hi int16-split
    compaction of 4096 token gathers from a host-concatenated
    [vocab, 3*m] bf16 table. Exact (16-aligned) static sizes per stream
    (max over cores) instead of per-chunk 128-rounding: ~3.5% padding.
  - Selection matrices generated ON-CHIP per 128-slot group via
    tensor_scalar(is_equal) of an iota row-constant against per-slot
    mem-ids (uploaded, tiny). Word-sum + slot->mem routing via PE
    matmuls accumulating in PSUM, two phases (lo then hi) per mem-chunk
    with an SBUF add between, so only ~2 chunk accumulators live at once.
  - E0T/E1T transposes inline as chunks complete; hop-0 scores, renorm,
    o-sums and u-update inline per batch as its 3 streams finish.
  - log(sum_exp) via exponent-extraction + cubic log2 poly on DVE
    (no Ln activation-table loads; Exp table loaded once at ramp).
The log-softmax renorms reduce to per-row affine transforms:
  renorm_q: p = (s - min(s)) / (sum(s) - 512*min(s) + 512e-8)
  renorm_a: p = (s - lse)   / (512*lse - sum(s)),  lse = logsumexp(s)
"""
import sys

for _p in ("/opt/pypackages", "/opt/trn_rl_repo"):
    if _p not in sys.path:
        sys.path.insert(0, _p)

import numpy as np
import ml_dtypes

import concourse.bass as bass
import concourse.bacc as bacc
import concourse.mybir as mybir
import concourse.tile as tile
from concourse.bass_utils import run_bass_kernel_spmd

BF = ml_dtypes.bfloat16

# problem constants
B, NMEM, NW, QLEN, NCH, CLEN = 32, 512, 8, 32, 8, 8
VOCAB, M = 50000, 256
EMB = 3 * M           # 768
NCORES = 8
BL = B // NCORES      # 4 batch per core
SPLIT = 32768         # int16 index split point
PIECE = 512           # slots per dma_gather piece
NSTREAM = 3 * BL      # 12 streams per core, s = 3*b + tau

# log2 cubic poly on m in [1,2): log2(m) ~ C3*m^3 + C2*m^2 + C1*m + C0
_LOGC = None

_cache = {}
TRACE = False
DEBUG = False
LAST_RESULTS = None


def _log2_poly():
    global _LOGC
    if _LOGC is None:
        m = np.linspace(1.0, 2.0, 4097)
        _LOGC = np.polyfit(m, np.log2(m), 3)   # [C3, C2, C1, C0]
    return _LOGC


def _align16(n):
    return -(-int(n) // 16) * 16


def _wrap_idx16(stream):
    """stream (len multiple of 16) -> [128, len/16] int16 wrapped layout:
    index i lives at [i % 16, i // 16], replicated across the 8 groups of 16
    partitions."""
    n = len(stream)
    c = n // 16
    arr = np.zeros((16, c), dtype=np.uint16)
    arr[:, :] = stream.astype(np.uint16).reshape(c, 16).T
    return np.tile(arr, (8, 1)).view(np.int16)


def _stream_tokens(subjects, relations, objects, b, tau):
    toks = (subjects, relations, objects)[tau]
    return toks[b].reshape(-1)     # [4096] mem-major


def _build_structure(subjects, relations, objects):
    """Static (all-core-union) structure: per-stream lo/hi sizes and
    group -> sorted chunk list."""
    LO, HI, gmap = [], [], []
    for s in range(NSTREAM):
        b, tau = divmod(s, 3)
        max_lo = max_hi = 0
        per_core = []
        for core in range(NCORES):
            t = _stream_tokens(subjects[core * BL:(core + 1) * BL],
                               relations[core * BL:(core + 1) * BL],
                               objects[core * BL:(core + 1) * BL], b, tau)
            lo_m = t < SPLIT
            mems = np.arange(4096) // NW
            lo_mm, hi_mm = mems[lo_m], mems[~lo_m]
            per_core.append((lo_mm, hi_mm))
            max_lo = max(max_lo, len(lo_mm))
            max_hi = max(max_hi, len(hi_mm))
        lo_c, hi_c = _align16(max_lo), _align16(max_hi)
        nglo = -(-lo_c // 128)
        nghi = -(-hi_c // 128)
        gm = [set() for _ in range(nglo + nghi)]
        for lo_mm, hi_mm in per_core:
            for g in range(nglo):
                seg = lo_mm[128 * g:128 * (g + 1)]
                if len(seg):
                    gm[g].update(np.unique(seg // 128).tolist())
            for g in range(nghi):
                seg = hi_mm[128 * g:128 * (g + 1)]
                if len(seg):
                    gm[nglo + g].update(np.unique(seg // 128).tolist())
        LO.append(lo_c)
        HI.append(hi_c)
        gmap.append(tuple(tuple(sorted(x)) for x in gm))
    return tuple(LO), tuple(HI), tuple(gmap)


def _build_core_data(subjects, relations, objects, LO, HI):
    """Per-core idx16 (padded to CMAX cols) and memid (padded to NGMAX)."""
    cmax = max((LO[s] + HI[s]) // 16 for s in range(NSTREAM))
    ngmax = max(-(-LO[s] // 128) - (-HI[s] // 128) for s in range(NSTREAM))
    idx16 = np.zeros((NSTREAM, 128, cmax), dtype=np.int16)
    memid = np.full((NSTREAM, 128, ngmax), 9999.0, dtype=np.float32)
    for s in range(NSTREAM):
        b, tau = divmod(s, 3)
        t = _stream_tokens(subjects, relations, objects, b, tau)
        lo_m = t < SPLIT
        mems = np.arange(4096) // NW
        lo_v, lo_mm = t[lo_m], mems[lo_m]
        hi_v, hi_mm = t[~lo_m] - SPLIT, mems[~lo_m]
        lo_c, hi_c = LO[s], HI[s]
        if len(lo_v) > lo_c or len(hi_v) > hi_c:
            raise OverflowError("stream overflow")
        stream = np.zeros(lo_c + hi_c, dtype=np.int64)
        stream[:len(lo_v)] = lo_v
        stream[lo_c:lo_c + len(hi_v)] = hi_v
        idx16[s, :, :(lo_c + hi_c) // 16] = _wrap_idx16(stream)
        nglo = -(-lo_c // 128)
        for g in range(nglo):
            seg = lo_mm[128 * g:128 * (g + 1)]
            memid[s, :len(seg), g] = seg.astype(np.float32)
        for g in range(-(-hi_c // 128)):
            seg = hi_mm[128 * g:128 * (g + 1)]
            memid[s, :len(seg), nglo + g] = seg.astype(np.float32)
    return idx16, memid, cmax, ngmax


def _pieces(size):
    """Split a section of `size` slots into gather pieces <= PIECE."""
    out = []
    off = 0
    while off < size:
        n = min(PIECE, size - off)
        out.append((off, n))
        off += n
    return out


def _runs(gmap_s, nglo, ngroups):
    """Per (chunk, phase) -> (first_group, last_group) over the union map.
    phase 0 = lo (groups < nglo), phase 1 = hi."""
    runs = {}
    for g, cs in enumerate(gmap_s):
        ph = 0 if g < nglo else 1
        for c in cs:
            k = (c, ph)
            if k not in runs:
                runs[k] = [g, g]
            runs[k][0] = min(runs[k][0], g)
            runs[k][1] = max(runs[k][1], g)
    return runs


def _build_program(LO, HI, gmap, cmax, ngmax):
    key = (LO, HI, gmap, DEBUG)
    if key in _cache:
        return _cache[key]

    f32 = mybir.dt.float32
    bf16 = mybir.dt.bfloat16
    C3, C2, C1, C0 = [float(c) for c in _log2_poly()]
    LN2 = float(np.log(2.0))

    nc = bacc.Bacc("TRN2", target_bir_lowering=False, debug=False)
    a_cat = nc.dram_tensor("a_cat", [VOCAB, EMB], bf16, kind="ExternalInput")
    b_tab = nc.dram_tensor("b_tab", [VOCAB, EMB], bf16, kind="ExternalInput")
    ut_d = nc.dram_tensor("ut", [EMB, EMB], bf16, kind="ExternalInput")
    vt_d = nc.dram_tensor("vt", [EMB, EMB], bf16, kind="ExternalInput")
    w_d = nc.dram_tensor("w", [EMB, EMB], bf16, kind="ExternalInput")
    idx16_d = nc.dram_tensor("idx16", [NSTREAM, 128, cmax], mybir.dt.int16,
                             kind="ExternalInput")
    memid_d = nc.dram_tensor("memid", [NSTREAM, 128, ngmax], f32,
                             kind="ExternalInput")
    iota_d = nc.dram_tensor("iota", [128, NMEM], mybir.dt.int16,
                           kind="ExternalInput")
    ones3_d = nc.dram_tensor("ones3", [3, 128, 128], bf16, kind="ExternalInput")
    maskq_d = nc.dram_tensor("maskq", [128, 1], f32, kind="ExternalInput")
    identf_d = nc.dram_tensor("identf", [128, 128], f32, kind="ExternalInput")
    identb_d = nc.dram_tensor("identb", [128, 128], bf16, kind="ExternalInput")
    idxua_d = nc.dram_tensor("idxua", [128, 3], mybir.dt.int32,
                             kind="ExternalInput")
    out_d = nc.dram_tensor("pred", [BL, NCH], f32, kind="ExternalOutput")
    if DEBUG:
        e_dbg = nc.dram_tensor("e_dbg", [NSTREAM, 128, 4, EMB], bf16,
                               kind="ExternalOutput")
        e0t_dbg = nc.dram_tensor("e0t_dbg", [BL, 128, 6, NMEM], bf16,
                                 kind="ExternalOutput")
        e1t_dbg = nc.dram_tensor("e1t_dbg", [BL, 128, 6, NMEM], bf16,
                                 kind="ExternalOutput")
        x_dbg = nc.dram_tensor("x_dbg", [128, EMB], f32,
                               kind="ExternalOutput")
        p_dbg = nc.dram_tensor("p_dbg", [128, NMEM], bf16,
                               kind="ExternalOutput")

    with tile.TileContext(nc) as tc:
        with (
            tc.tile_pool(name="const", bufs=1) as constp,
            tc.tile_pool(name="state", bufs=1) as statep,
            tc.tile_pool(name="selp1", bufs=16) as selp1,
            tc.tile_pool(name="selp2", bufs=4) as selp2,
            tc.tile_pool(name="wsp", bufs=2, space="PSUM") as wsp,
            tc.tile_pool(name="tpps", bufs=1, space="PSUM") as tpps,
            tc.tile_pool(name="yps", bufs=1, space="PSUM") as ypsp,
            tc.tile_pool(name="ops", bufs=1, space="PSUM") as opsp,
            tc.tile_pool(name="hps", bufs=1, space="PSUM") as hpsp,
        ):
            # ---- small consts first (gathers depend on idx/memid) ----
            idx_sb = [constp.tile([128, cmax], mybir.dt.int16, tag=f"idx{i}",
                                  name=f"idx{i}") for i in range(2)]
            mem_sb = [constp.tile([128, ngmax], f32, tag=f"mem{s}",
                                  name=f"mem{s}") for s in range(NSTREAM)]
            nc.sync.dma_start(out=idx_sb[0][:], in_=idx16_d[0])
            nc.sync.dma_start(out=mem_sb[0][:], in_=memid_d[0])
            iota = constp.tile([128, NMEM], mybir.dt.int16, tag="iota")
            nc.sync.dma_start(out=iota[:], in_=iota_d[:])
            identb = constp.tile([128, 128], bf16, tag="identb")
            nc.sync.dma_start(out=identb[:], in_=identb_d[:])

            # persistent state
            X = statep.tile([128, EMB], f32, tag="X")
            E = [statep.tile([128, 4, EMB], bf16, tag=f"E{s}", name=f"E{s}")
                 for s in range(NSTREAM)]
            E0T = [statep.tile([128, 6, NMEM], bf16, tag=f"E0T{b}",
                               name=f"E0T{b}") for b in range(BL)]
            E1T = [statep.tile([128, 6, NMEM], bf16, tag=f"E1T{b}",
                               name=f"E1T{b}") for b in range(BL)]
            G = [statep.tile([128, 4, EMB], bf16, tag=f"G{i}", name=f"G{i}")
                 for i in range(6)]
            S = hpsp.tile([128, NMEM], f32, tag="S")
            P = statep.tile([128, NMEM], bf16, tag="P")
            PT = statep.tile([128, 4, 128], bf16, tag="PT")
            o_sb = statep.tile([128, EMB], bf16, tag="o_sb")
            # renorm scalars [128,1]
            sc = {n: statep.tile([128, 1], f32, tag=n, name=n)
                  for n in ("mx", "mn", "sm", "negmx", "se", "lse", "ef",
                            "mf", "t1", "Av", "bq", "ba", "Bv", "invb")}

            # memset gather buffers once (NaN-safety for stale group tails)
            for i in range(6):
                nc.any.memset(G[i][:], 0.0)
            nc.vector.memset(S[:], 0.0)
            nc.any.memset(P[:], 0.0)

            # ---- remaining consts ----
            nc.sync.dma_start(out=idx_sb[1][:], in_=idx16_d[1])
            for s in range(1, NSTREAM):
                nc.sync.dma_start(out=mem_sb[s][:], in_=memid_d[s])
            identf = constp.tile([128, 128], f32, tag="identf")
            nc.sync.dma_start(out=identf[:], in_=identf_d[:])
            maskq = constp.tile([128, 1], f32, tag="maskq")
            nc.sync.dma_start(out=maskq[:], in_=maskq_d[:])
            ones3 = [constp.tile([128, 128], bf16, tag=f"ones{i}",
                                 name=f"ones{i}") for i in range(3)]
            for i in range(3):
                nc.sync.dma_start(out=ones3[i][:], in_=ones3_d[i])
            idxua = constp.tile([128, 3], mybir.dt.int32, tag="idxua")
            nc.sync.dma_start(out=idxua[:], in_=idxua_d[:])
            ut_sb = constp.tile([128, 6, EMB], bf16, tag="ut")
            vt_sb = constp.tile([128, 6, EMB], bf16, tag="vt")
            w_sb = constp.tile([128, 6, EMB], bf16, tag="w")
            for t_sb, t_d in ((ut_sb, ut_d), (vt_sb, vt_d), (w_sb, w_d)):
                nc.sync.dma_start(
                    out=t_sb[:],
                    in_=t_d[:].rearrange("(j p) d -> p j d", p=128))

            # ---- init: u and a from B_table -> X; preload Exp table ----
            # stage the 3 B-table gathers in G[0] (free until stream 0)
            for i in range(3):
                nc.gpsimd.indirect_dma_start(
                    out=G[0][:, i, :], out_offset=None, in_=b_tab[:],
                    in_offset=bass.IndirectOffsetOnAxis(
                        ap=idxua[:, i:i + 1], axis=0))
            ps0 = wsp.tile([128, EMB], f32, tag="wsp", name="ps0")
            for i in range(3):
                nc.tensor.matmul(out=ps0[:, 0:512], lhsT=ones3[i][:],
                                 rhs=G[0][:, i, 0:512],
                                 start=(i == 0), stop=(i == 2))
                nc.tensor.matmul(out=ps0[:, 512:768], lhsT=ones3[i][:],
                                 rhs=G[0][:, i, 512:768],
                                 start=(i == 0), stop=(i == 2))
            nc.vector.tensor_copy(out=X[:, 0:512], in_=ps0[:, 0:512])
            nc.vector.tensor_copy(out=X[:, 512:768], in_=ps0[:, 512:768])
            # warm the Exp activation table (only table ever used)
            nc.scalar.activation(out=sc["ef"][:], in_=maskq[:],
                                 func=mybir.ActivationFunctionType.Exp)

            # ---- helpers ----
            def xt_extract(src_sb, ident, xq, xa, dt):
                """Transpose [128, 768] src into per-b q/a column tiles."""
                for j in range(6):
                    tp = tpps.tile([128, 128], dt, tag="tp")
                    nc.tensor.transpose(
                        out=tp[:], in_=src_sb[:, 128 * j:128 * j + 128],
                        identity=ident[:])
                    v = tp[:].rearrange("p (b n) -> p b n", b=4)
                    nc.vector.tensor_copy(out=xq[:, j, :], in_=v[:, :, 0])
                    nc.vector.tensor_copy(out=xa[:, j, :], in_=v[:, :, 1:9])

            def y_project(xq, xa, ysb):
                for i in range(6):
                    y36 = ypsp.tile([128, 36], f32, tag="y36", name="y36")
                    for j in range(6):
                        nc.tensor.matmul(
                            out=y36[:, 0:4],
                            lhsT=ut_sb[:, j, 128 * i:128 * i + 128],
                            rhs=xq[:, j, :], start=(j == 0), stop=(j == 5))
                    for j in range(6):
                        nc.tensor.matmul(
                            out=y36[:, 4:36],
                            lhsT=vt_sb[:, j, 128 * i:128 * i + 128],
                            rhs=xa[:, j, :], start=(j == 0), stop=(j == 5))
                    yv = ysb[:, i, :].rearrange("p (b n) -> p b n", b=4)
                    nc.vector.tensor_copy(out=yv[:, :, 0], in_=y36[:, 0:4])
                    nc.vector.tensor_copy(out=yv[:, :, 1:9], in_=y36[:, 4:36])

            def renorm(rows, n):
                """Renorm S[rows:rows+n] -> P[rows:rows+n] (bf16)."""
                rs = slice(rows, rows + n)
                nc.vector.tensor_reduce(out=sc["mx"][rs], in_=S[rs, :],
                                        axis=mybir.AxisListType.X,
                                        op=mybir.AluOpType.max)
                nc.vector.tensor_scalar(out=sc["negmx"][rs], in0=sc["mx"][rs],
                                        scalar1=-1.0, scalar2=None,
                                        op0=mybir.AluOpType.mult)
                nc.scalar.activation(
                    out=P[rs, :], in_=S[rs, :],
                    func=mybir.ActivationFunctionType.Exp,
                    bias=sc["negmx"][rs], scale=1.0, accum_out=sc["se"][rs])
                nc.vector.tensor_reduce(out=sc["mn"][rs], in_=S[rs, :],
                                        axis=mybir.AxisListType.X,
                                        op=mybir.AluOpType.min)
                nc.vector.tensor_reduce(out=sc["sm"][rs], in_=S[rs, :],
                                        axis=mybir.AxisListType.X,
                                        op=mybir.AluOpType.add)
                # lse = mx + ln2 * log2(se); log2 via exponent + cubic poly
                se_i = sc["se"][rs].bitcast(mybir.dt.int32)
                ef_i = sc["ef"][rs].bitcast(mybir.dt.int32)
                nc.vector.tensor_scalar(out=ef_i, in0=se_i,
                                        scalar1=23, scalar2=None,
                                        op0=mybir.AluOpType.logical_shift_right)
                nc.vector.tensor_copy(out=sc["ef"][rs], in_=ef_i)
                mf_i = sc["mf"][rs].bitcast(mybir.dt.int32)
                nc.vector.tensor_scalar(out=mf_i, in0=se_i,
                                        scalar1=0x007FFFFF,
                                        scalar2=0x3F800000,
                                        op0=mybir.AluOpType.bitwise_and,
                                        op1=mybir.AluOpType.bitwise_or)
                m = sc["mf"][rs]
                t1 = sc["t1"][rs]
                nc.vector.tensor_scalar(out=t1, in0=m, scalar1=C3, scalar2=C2,
                                        op0=mybir.AluOpType.mult,
                                        op1=mybir.AluOpType.add)
                nc.vector.tensor_tensor(out=t1, in0=t1, in1=m,
                                        op=mybir.AluOpType.mult)
                nc.vector.tensor_scalar(out=t1, in0=t1, scalar1=C1,
                                        scalar2=None,
                                        op0=mybir.AluOpType.add)
                nc.vector.tensor_tensor(out=t1, in0=t1, in1=m,
                                        op=mybir.AluOpType.mult)
                nc.vector.tensor_scalar(out=t1, in0=t1, scalar1=C0 - 127.0,
                                        scalar2=None,
                                        op0=mybir.AluOpType.add)
                nc.vector.tensor_tensor(out=t1, in0=t1, in1=sc["ef"][rs],
                                        op=mybir.AluOpType.add)
                nc.vector.tensor_scalar(out=sc["lse"][rs], in0=t1,
                                        scalar1=LN2, scalar2=sc["mx"][rs],
                                        op0=mybir.AluOpType.mult,
                                        op1=mybir.AluOpType.add)
                # A = lse + maskq*(mn - lse)
                nc.vector.tensor_tensor(out=t1, in0=sc["mn"][rs],
                                        in1=sc["lse"][rs],
                                        op=mybir.AluOpType.subtract)
                nc.vector.tensor_tensor(out=t1, in0=t1, in1=maskq[rs],
                                        op=mybir.AluOpType.mult)
                nc.vector.tensor_tensor(out=sc["Av"][rs], in0=sc["lse"][rs],
                                        in1=t1, op=mybir.AluOpType.add)
                # Bq = sm - 512*mn + 512e-8 ; Ba = 512*lse - sm
                nc.vector.tensor_scalar(out=sc["bq"][rs], in0=sc["mn"][rs],
                                        scalar1=-512.0, scalar2=512e-8,
                                        op0=mybir.AluOpType.mult,
                                        op1=mybir.AluOpType.add)
                nc.vector.tensor_tensor(out=sc["bq"][rs], in0=sc["bq"][rs],
                                        in1=sc["sm"][rs],
                                        op=mybir.AluOpType.add)
                nc.vector.tensor_scalar(out=sc["ba"][rs], in0=sc["lse"][rs],
                                        scalar1=512.0, scalar2=None,
                                        op0=mybir.AluOpType.mult)
                nc.vector.tensor_tensor(out=sc["ba"][rs], in0=sc["ba"][rs],
                                        in1=sc["sm"][rs],
                                        op=mybir.AluOpType.subtract)
                nc.vector.tensor_tensor(out=sc["Bv"][rs], in0=sc["bq"][rs],
                                        in1=sc["ba"][rs],
                                        op=mybir.AluOpType.subtract)
                nc.vector.tensor_tensor(out=sc["Bv"][rs], in0=sc["Bv"][rs],
                                        in1=maskq[rs],
                                        op=mybir.AluOpType.mult)
                nc.vector.tensor_tensor(out=sc["Bv"][rs], in0=sc["Bv"][rs],
                                        in1=sc["ba"][rs],
                                        op=mybir.AluOpType.add)
                nc.vector.reciprocal(out=sc["invb"][rs], in_=sc["Bv"][rs])
                for kk in range(4):
                    nc.vector.tensor_scalar(
                        out=P[rs, 128 * kk:128 * kk + 128],
                        in0=S[rs, 128 * kk:128 * kk + 128],
                        scalar1=sc["Av"][rs],
                        scalar2=sc["invb"][rs],
                        op0=mybir.AluOpType.subtract,
                        op1=mybir.AluOpType.mult)

            # hop-0 Y from initial X (runs during early gathers)
            xtq0 = statep.tile([128, 6, 4], bf16, tag="xtq0")
            xta0 = statep.tile([128, 6, 32], bf16, tag="xta0")
            ysb0 = statep.tile([128, 6, 36], bf16, tag="ysb0")
            xt_extract(X, identf, xtq0, xta0, f32)
            y_project(xtq0, xta0, ysb0)

            # ---- gather streams + word-sum + inline hop 0 ----
            piece_ctr = 0
            pending = []

            def act_copy(out, in_):
                nc.scalar.activation(
                    out=out, in_=in_,
                    func=mybir.ActivationFunctionType.Copy)
            for s in range(NSTREAM):
                b, tau = divmod(s, 3)
                lo_c, hi_c = LO[s], HI[s]
                nglo = -(-lo_c // 128)
                ng = nglo + (-(-hi_c // 128))
                runs = _runs(gmap[s], nglo, ng)
                # chunk psum tiles keyed (c, phase)
                pst = {}
                done_groups = {}

                if s + 1 < NSTREAM:
                    nc.sync.dma_start(out=idx_sb[(s + 1) % 2][:],
                                      in_=idx16_d[s + 1])
                sections = [(0, lo_c, 0, a_cat[:, :]),
                            (nglo, hi_c, lo_c, a_cat[SPLIT:, :])]
                for gbase, size, stream_off, src in sections:
                    for off, n in _pieces(size):
                        gt = G[piece_ctr % 6]
                        piece_ctr += 1
                        npg = -(-n // 128)
                        nc.gpsimd.dma_gather(
                            gt[:, 0:npg, :], src,
                            idx_sb[s % 2][:, (stream_off + off) // 16:
                                          (stream_off + off + n) // 16],
                            n, n, EMB)
                        if pending:
                            pending.pop(0)()
                        g0 = gbase + off // 128
                        sels = {}
                        for gl in range(npg):
                            g = g0 + gl
                            cs = gmap[s][g]
                            if not cs:
                                continue
                            c0, c1 = cs[0], cs[-1]
                            w = 128 * (c1 - c0 + 1)
                            if w > 128:
                                sel = selp2.tile([128, 256], bf16, tag="sl2",
                                                 name="sl2")
                            else:
                                sel = selp1.tile([128, 128], bf16, tag="sl1",
                                                 name="sl1")
                            nc.vector.tensor_scalar(
                                out=sel[:, 0:w],
                                in0=iota[:, 128 * c0:128 * c0 + w],
                                scalar1=mem_sb[s][:, g:g + 1], scalar2=None,
                                op0=mybir.AluOpType.is_equal)
                            sels[gl] = (sel, c0)
                        for gl in range(npg):
                            g = g0 + gl
                            cs = gmap[s][g]
                            if not cs:
                                continue
                            sel, c0 = sels[gl]
                            ph = 0 if g < nglo else 1
                            for c in cs:
                                k = (c, ph)
                                if k not in pst:
                                    pst[k] = wsp.tile([128, EMB], f32,
                                                      tag="wsp", name="pw")
                                pw = pst[k]
                                pa, pb = pw[:, 0:512], pw[:, 512:768]
                                lhs = sel[:, 128 * (c - c0):
                                          128 * (c - c0) + 128]
                                first = runs[k][0] == g
                                last = runs[k][1] == g
                                nc.tensor.matmul(
                                    out=pa, lhsT=lhs, rhs=gt[:, gl, 0:512],
                                    start=first, stop=last)
                                nc.tensor.matmul(
                                    out=pb, lhsT=lhs,
                                    rhs=gt[:, gl, 512:768],
                                    start=first, stop=last)
                                if last:
                                    done_groups[k] = True
                                    pst.pop(k)
                                    if ph == 0:
                                        act_copy(E[s][:, c, 0:512], pa)
                                        act_copy(E[s][:, c, 512:768], pb)
                                    else:
                                        nc.vector.tensor_tensor(
                                            out=E[s][:, c, 0:512],
                                            in0=E[s][:, c, 0:512], in1=pa,
                                            op=mybir.AluOpType.add)
                                        nc.vector.tensor_tensor(
                                            out=E[s][:, c, 512:768],
                                            in0=E[s][:, c, 512:768],
                                            in1=pb,
                                            op=mybir.AluOpType.add)
                                        # inline transposes for E0T / E1T
                                        for q in range(2):
                                            tp = tpps.tile([128, 128], bf16,
                                                           tag="tp")
                                            nc.tensor.transpose(
                                                out=tp[:],
                                                in_=E[s][:, c, 128 * q:
                                                         128 * q + 128],
                                                identity=identb[:])
                                            act_copy(
                                                E0T[b][:, 2 * tau + q,
                                                       128 * c:
                                                       128 * c + 128],
                                                tp[:])
                                        for q in range(2):
                                            tp = tpps.tile([128, 128], bf16,
                                                           tag="tp")
                                            nc.tensor.transpose(
                                                out=tp[:],
                                                in_=E[s][:, c, 256 + 128 * q:
                                                         256 + 128 * q + 128],
                                                identity=identb[:])
                                            act_copy(
                                                E1T[b][:, 2 * tau + q,
                                                       128 * c:
                                                       128 * c + 128],
                                                tp[:])

                # ---- inline hop 0 for batch b once its 3 streams done ----
                if tau == 2:
                    def hop0_scores(b=b):
                        r0 = 32 * b
                        for j in range(6):
                            nc.tensor.matmul(
                                out=S[r0:r0 + 9, :],
                                lhsT=ysb0[:, j, 9 * b:9 * b + 9],
                                rhs=E0T[b][:, j, :], start=(j == 0),
                                stop=(j == 5), tile_position=(0, r0))

                    def hop0_renorm(b=b):
                        renorm(32 * b, 9)

                    def hop0_pt(b=b):
                        r0 = 32 * b
                        for k in range(4):
                            tp = tpps.tile([128, 128], bf16, tag="tp")
                            nc.tensor.transpose(
                                out=tp[:],
                                in_=P[:, 128 * k:128 * k + 128],
                                identity=identb[:])
                            nc.vector.tensor_copy(out=PT[:, k, r0:r0 + 9],
                                                  in_=tp[:, r0:r0 + 9])

                    def hop0_o(b=b, t2=0):
                        r0 = 32 * b
                        if t2 == 0:
                            odst = S[r0:r0 + 9, 0:256]
                        elif t2 == 1:
                            odst = S[r0:r0 + 9, 256:512]
                        else:
                            ot = opsp.tile([128, 256], f32, tag="ot",
                                           name="ot")
                            hop0_o.ot = ot
                            odst = ot[r0:r0 + 9, :]
                        for k in range(4):
                            nc.tensor.matmul(
                                out=odst,
                                lhsT=PT[:, k, r0:r0 + 9],
                                rhs=E[3 * b + t2][:, k, 256:512],
                                start=(k == 0), stop=(k == 3),
                                tile_position=(0, r0))
                        nc.vector.tensor_tensor(
                            out=X[r0:r0 + 9, 256 * t2:256 * t2 + 256],
                            in0=X[r0:r0 + 9, 256 * t2:256 * t2 + 256],
                            in1=odst,
                            op=mybir.AluOpType.add)

                    if b < BL - 1:
                        pending.extend([
                            hop0_scores, hop0_renorm, hop0_pt,
                            lambda b=b: hop0_o(b, 0),
                            lambda b=b: hop0_o(b, 1),
                            lambda b=b: hop0_o(b, 2)])
                    else:
                        hop0_scores()
                        hop0_renorm()
                        hop0_pt()
                        for t2 in range(3):
                            hop0_o(b, t2)

            while pending:
                pending.pop(0)()

            if DEBUG:
                for s in range(NSTREAM):
                    nc.sync.dma_start(out=e_dbg[s], in_=E[s][:])
                for b in range(BL):
                    nc.sync.dma_start(out=e0t_dbg[b], in_=E0T[b][:])
                    nc.sync.dma_start(out=e1t_dbg[b], in_=E1T[b][:])
                nc.sync.dma_start(out=x_dbg[:], in_=X[:])
                nc.sync.dma_start(out=p_dbg[:], in_=P[:])

            # ---------------- hop 1 + final ----------------
            xtq1 = statep.tile([128, 6, 4], bf16, tag="xtq1")
            xta1 = statep.tile([128, 6, 32], bf16, tag="xta1")
            ysb1 = statep.tile([128, 6, 36], bf16, tag="ysb1")
            xt_extract(X, identf, xtq1, xta1, f32)
            y_project(xtq1, xta1, ysb1)
            for b in range(BL):
                r0 = 32 * b
                for j in range(6):
                    nc.tensor.matmul(
                        out=S[r0:r0 + 9, :], lhsT=ysb1[:, j, 9 * b:9 * b + 9],
                        rhs=E1T[b][:, j, :], start=(j == 0), stop=(j == 5),
                        tile_position=(0, r0))
            renorm(0, 128)
            for k in range(4):
                tp = tpps.tile([128, 128], bf16, tag="tp")
                nc.tensor.transpose(out=tp[:], in_=P[:, 128 * k:128 * k + 128],
                                    identity=identb[:])
                nc.vector.tensor_copy(out=PT[:, k, :], in_=tp[:])
            ot1 = opsp.tile([128, 256], f32, tag="ot", name="ot1")
            nc.vector.memset(ot1[:], 0.0)
            for b in range(BL):
                r0 = 32 * b
                odst = [S[r0:r0 + 9, 0:256], S[r0:r0 + 9, 256:512],
                        ot1[r0:r0 + 9, :]]
                for t2 in range(3):
                    for k in range(4):
                        nc.tensor.matmul(
                            out=odst[t2],
                            lhsT=PT[:, k, r0:r0 + 9],
                            rhs=E[3 * b + t2][:, k, 512:768],
                            start=(k == 0), stop=(k == 3),
                            tile_position=(0, r0))
            nc.vector.tensor_copy(out=o_sb[:, 0:256], in_=S[:, 0:256])
            nc.vector.tensor_copy(out=o_sb[:, 256:512], in_=S[:, 256:512])
            nc.vector.tensor_copy(out=o_sb[:, 512:768], in_=ot1[:])

            # final bilinear form: pred[b,c] = o_q[b] . (W @ o_a[c,b])
            otq = statep.tile([128, 6, 4], bf16, tag="otq")
            ota = statep.tile([128, 6, 32], bf16, tag="ota")
            xt_extract(o_sb, identb, otq, ota, bf16)
            wq = statep.tile([128, 6, 4], bf16, tag="wq")
            for i in range(6):
                wqp = ypsp.tile([128, 36], f32, tag="y36", name="wqp")
                for j in range(6):
                    nc.tensor.matmul(
                        out=wqp[:, 0:4],
                        lhsT=w_sb[:, j, 128 * i:128 * i + 128],
                        rhs=otq[:, j, :], start=(j == 0), stop=(j == 5))
                nc.vector.tensor_copy(out=wq[:, i, :], in_=wqp[:, 0:4])
            predp36 = ypsp.tile([128, 36], f32, tag="y36", name="predp36")
            predp = predp36[:, 0:NCH]
            pred_sb = statep.tile([128, NCH], f32, tag="pred_sb")
            for b in range(BL):
                for i in range(6):
                    nc.tensor.matmul(
                        out=predp36[32 * b:32 * b + 1, 0:NCH],
                        lhsT=wq[:, i, b:b + 1],
                        rhs=ota[:, i, 8 * b:8 * b + 8],
                        start=(i == 0), stop=(i == 5),
                        tile_position=(0, 32 * b))
                nc.vector.tensor_copy(out=pred_sb[32 * b:32 * b + 1, :],
                                      in_=predp36[32 * b:32 * b + 1, 0:NCH])
                nc.sync.dma_start(out=out_d[b:b + 1, :],
                                  in_=pred_sb[32 * b:32 * b + 1, :])

    nc.compile()
    _cache[key] = nc
    return nc


def prepare(subjects, relations, objects, ques, answerChoices,
            A_tables, B_table, U, V, W):
    subjects = np.asarray(subjects).astype(np.int64)
    relations = np.asarray(relations).astype(np.int64)
    objects = np.asarray(objects).astype(np.int64)
    ques = np.asarray(ques).astype(np.int64)
    answerChoices = np.asarray(answerChoices).astype(np.int64)
    A_tables = np.asarray(A_tables, dtype=np.float32)
    B_table = np.asarray(B_table, dtype=np.float32)

    # shared (core-independent) device data
    a_cat = np.concatenate([A_tables[0], A_tables[1], A_tables[2]],
                           axis=1).astype(BF)
    b_bf = B_table.astype(BF)
    ut = np.ascontiguousarray(np.asarray(U, dtype=np.float32).T).astype(BF)
    vt = np.ascontiguousarray(np.asarray(V, dtype=np.float32).T).astype(BF)
    w_bf = np.ascontiguousarray(np.asarray(W, dtype=np.float32)).astype(BF)
    identf = np.eye(128, dtype=np.float32)
    identb = np.eye(128, dtype=BF)
    maskq = np.zeros((128, 1), dtype=np.float32)
    maskq[0::32] = 1.0
    iota = np.tile(np.arange(NMEM, dtype=np.int16), (128, 1))
    # init placement matrices (state row = 32*b + 0 for u, +1+c for choices)
    ones3 = np.zeros((3, 128, 128), dtype=BF)
    p = np.arange(128)
    ones3[0, p, 32 * (p // 32)] = 1.0                        # u rows
    ones3[1, p, 32 * (p // 64) + 1 + (p // 8) % 8] = 1.0     # a, b in {0,1}
    ones3[2, p, 32 * (2 + p // 64) + 1 + (p // 8) % 8] = 1.0  # a, b in {2,3}

    LO, HI, gmap = _build_structure(subjects, relations, objects)
    cmax = max((LO[s] + HI[s]) // 16 for s in range(NSTREAM))
    ngmax = max(-(-LO[s] // 128) - (-HI[s] // 128) for s in range(NSTREAM))
    nc = _build_program(LO, HI, gmap, cmax, ngmax)

    in_maps = []
    for core in range(NCORES):
        sl = slice(core * BL, (core + 1) * BL)
        idx16, memid, _, _ = _build_core_data(
            subjects[sl], relations[sl], objects[sl], LO, HI)
        idxua = np.zeros((128, 3), dtype=np.int32)
        idxua[:, 0] = ques[sl][p // 32, p % 32]
        idxua[:, 1] = answerChoices[sl][p // 64, (p // 8) % 8, p % 8]
        idxua[:, 2] = answerChoices[sl][2 + p // 64, (p // 8) % 8, p % 8]
        in_maps.append(dict(
            a_cat=a_cat, b_tab=b_bf, ut=ut, vt=vt, w=w_bf,
            idx16=idx16, memid=memid, iota=iota, ones3=ones3, maskq=maskq,
            identf=identf, identb=identb, idxua=idxua))

    return nc, in_maps


def kernel(**inputs):
    nc, in_maps = prepare(**inputs)
    res = run_bass_kernel_spmd(nc, in_maps, list(range(NCORES)), trace=TRACE)
    global LAST_RESULTS
    LAST_RESULTS = res
    return np.concatenate([res.results[c]["pred"] for c in range(NCORES)],
                          axis=0).astype(np.float32)


# revision 36
# speedup vs baseline: 1.0317x; 1.0289x over previous
"""Trainium2 Bass kernel for the 2-hop key-value memory network.

v2 strategy: data-parallel over batch (B=32 -> 4 per core x 8 cores).
Per core:
  - 12 gather streams (b-outer, tau in s/r/o), each a lo/hi int16-split
    compaction of 4096 token gathers from a host-concatenated
    [vocab, 3*m] bf16 table. Exact (16-aligned) static sizes per stream
    (max over cores) instead of per-chunk 128-rounding: ~3.5% padding.
  - Selection matrices generated ON-CHIP per 128-slot group via
    tensor_scalar(is_equal) of an iota row-constant against per-slot
    mem-ids (uploaded, tiny). Word-sum + slot->mem routing via PE
    matmuls accumulating in PSUM, two phases (lo then hi) per mem-chunk
    with an SBUF add between, so only ~2 chunk accumulators live at once.
  - E0T/E1T transposes inline as chunks complete; hop-0 scores, renorm,
    o-sums and u-update inline per batch as its 3 streams finish.
  - log(sum_exp) via exponent-extraction + cubic log2 poly on DVE
    (no Ln activation-table loads; Exp table loaded once at ramp).
The log-softmax renorms reduce to per-row affine transforms:
  renorm_q: p = (s - min(s)) / (sum(s) - 512*min(s) + 512e-8)
  renorm_a: p = (s - lse)   / (512*lse - sum(s)),  lse = logsumexp(s)
"""
import sys

for _p in ("/opt/pypackages", "/opt/trn_rl_repo"):
    if _p not in sys.path:
        sys.path.insert(0, _p)

import numpy as np
import ml_dtypes

import concourse.bass as bass
import concourse.bacc as bacc
import concourse.mybir as mybir
import concourse.tile as tile
from concourse.bass_utils import run_bass_kernel_spmd

BF = ml_dtypes.bfloat16

# problem constants
B, NMEM, NW, QLEN, NCH, CLEN = 32, 512, 8, 32, 8, 8
VOCAB, M = 50000, 256
EMB = 3 * M           # 768
NCORES = 8
BL = B // NCORES      # 4 batch per core
SPLIT = 32768         # int16 index split point
PIECE = 512           # slots per dma_gather piece
NSTREAM = 3 * BL      # 12 streams per core, s = 3*b + tau

# log2 cubic poly on m in [1,2): log2(m) ~ C3*m^3 + C2*m^2 + C1*m + C0
_LOGC = None

_cache = {}
TRACE = False
DEBUG = False
LAST_RESULTS = None


def _log2_poly():
    global _LOGC
    if _LOGC is None:
        m = np.linspace(1.0, 2.0, 4097)
        _LOGC = np.polyfit(m, np.log2(m), 3)   # [C3, C2, C1, C0]
    return _LOGC


def _align16(n):
    return -(-int(n) // 16) * 16


def _wrap_idx16(stream):
    """stream (len multiple of 16) -> [128, len/16] int16 wrapped layout:
    index i lives at [i % 16, i // 16], replicated across the 8 groups of 16
    partitions."""
    n = len(stream)
    c = n // 16
    arr = np.zeros((16, c), dtype=np.uint16)
    arr[:, :] = stream.astype(np.uint16).reshape(c, 16).T
    return np.tile(arr, (8, 1)).view(np.int16)


def _stream_tokens(subjects, relations, objects, b, tau):
    toks = (subjects, relations, objects)[tau]
    return toks[b].reshape(-1)     # [4096] mem-major


def _build_structure(subjects, relations, objects):
    """Static (all-core-union) structure: per-stream lo/hi sizes and
    group -> sorted chunk list."""
    LO, HI, gmap = [], [], []
    for s in range(NSTREAM):
        b, tau = divmod(s, 3)
        max_lo = max_hi = 0
        per_core = []
        for core in range(NCORES):
            t = _stream_tokens(subjects[core * BL:(core + 1) * BL],
                               relations[core * BL:(core + 1) * BL],
                               objects[core * BL:(core + 1) * BL], b, tau)
            lo_m = t < SPLIT
            mems = np.arange(4096) // NW
            lo_mm, hi_mm = mems[lo_m], mems[~lo_m]
            per_core.append((lo_mm, hi_mm))
            max_lo = max(max_lo, len(lo_mm))
            max_hi = max(max_hi, len(hi_mm))
        lo_c, hi_c = _align16(max_lo), _align16(max_hi)
        nglo = -(-lo_c // 128)
        nghi = -(-hi_c // 128)
        gm = [set() for _ in range(nglo + nghi)]
        for lo_mm, hi_mm in per_core:
            for g in range(nglo):
                seg = lo_mm[128 * g:128 * (g + 1)]
                if len(seg):
                    gm[g].update(np.unique(seg // 128).tolist())
            for g in range(nghi):
                seg = hi_mm[128 * g:128 * (g + 1)]
                if len(seg):
                    gm[nglo + g].update(np.unique(seg // 128).tolist())
        LO.append(lo_c)
        HI.append(hi_c)
        gmap.append(tuple(tuple(sorted(x)) for x in gm))
    return tuple(LO), tuple(HI), tuple(gmap)


def _build_core_data(subjects, relations, objects, LO, HI):
    """Per-core idx16 (padded to CMAX cols) and memid (padded to NGMAX)."""
    cmax = max((LO[s] + HI[s]) // 16 for s in range(NSTREAM))
    ngmax = max(-(-LO[s] // 128) - (-HI[s] // 128) for s in range(NSTREAM))
    idx16 = np.zeros((NSTREAM, 128, cmax), dtype=np.int16)
    memid = np.full((NSTREAM, 128, ngmax), 9999.0, dtype=np.float32)
    for s in range(NSTREAM):
        b, tau = divmod(s, 3)
        t = _stream_tokens(subjects, relations, objects, b, tau)
        lo_m = t < SPLIT
        mems = np.arange(4096) // NW
        lo_v, lo_mm = t[lo_m], mems[lo_m]
        hi_v, hi_mm = t[~lo_m] - SPLIT, mems[~lo_m]
        lo_c, hi_c = LO[s], HI[s]
        if len(lo_v) > lo_c or len(hi_v) > hi_c:
            raise OverflowError("stream overflow")
        stream = np.zeros(lo_c + hi_c, dtype=np.int64)
        stream[:len(lo_v)] = lo_v
        stream[lo_c:lo_c + len(hi_v)] = hi_v
        idx16[s, :, :(lo_c + hi_c) // 16] = _wrap_idx16(stream)
        nglo = -(-lo_c // 128)
        for g in range(nglo):
            seg = lo_mm[128 * g:128 * (g + 1)]
            memid[s, :len(seg), g] = seg.astype(np.float32)
        for g in range(-(-hi_c // 128)):
            seg = hi_mm[128 * g:128 * (g + 1)]
            memid[s, :len(seg), nglo + g] = seg.astype(np.float32)
    return idx16, memid, cmax, ngmax


def _pieces(size):
    """Split a section of `size` slots into gather pieces <= PIECE."""
    out = []
    off = 0
    while off < size:
        n = min(PIECE, size - off)
        out.append((off, n))
        off += n
    return out


def _runs(gmap_s, nglo, ngroups):
    """Per (chunk, phase) -> (first_group, last_group) over the union map.
    phase 0 = lo (groups < nglo), phase 1 = hi."""
    runs = {}
    for g, cs in enumerate(gmap_s):
        ph = 0 if g < nglo else 1
        for c in cs:
            k = (c, ph)
            if k not in runs:
                runs[k] = [g, g]
            runs[k][0] = min(runs[k][0], g)
            runs[k][1] = max(runs[k][1], g)
    return runs


def _build_program(LO, HI, gmap, cmax, ngmax):
    key = (LO, HI, gmap, DEBUG)
    if key in _cache:
        return _cache[key]

    f32 = mybir.dt.float32
    bf16 = mybir.dt.bfloat16
    C3, C2, C1, C0 = [float(c) for c in _log2_poly()]
    LN2 = float(np.log(2.0))

    nc = bacc.Bacc("TRN2", target_bir_lowering=False, debug=False)
    a_cat = nc.dram_tensor("a_cat", [VOCAB, EMB], bf16, kind="ExternalInput")
    b_tab = nc.dram_tensor("b_tab", [VOCAB, EMB], bf16, kind="ExternalInput")
    ut_d = nc.dram_tensor("ut", [EMB, EMB], bf16, kind="ExternalInput")
    vt_d = nc.dram_tensor("vt", [EMB, EMB], bf16, kind="ExternalInput")
    w_d = nc.dram_tensor("w", [EMB, EMB], bf16, kind="ExternalInput")
    idx16_d = nc.dram_tensor("idx16", [NSTREAM, 128, cmax], mybir.dt.int16,
                             kind="ExternalInput")
    memid_d = nc.dram_tensor("memid", [NSTREAM, 128, ngmax], f32,
                             kind="ExternalInput")
    iota_d = nc.dram_tensor("iota", [128, NMEM], mybir.dt.int16,
                           kind="ExternalInput")
    ones3_d = nc.dram_tensor("ones3", [3, 128, 128], bf16, kind="ExternalInput")
    maskq_d = nc.dram_tensor("maskq", [128, 1], f32, kind="ExternalInput")
    identf_d = nc.dram_tensor("identf", [128, 128], f32, kind="ExternalInput")
    identb_d = nc.dram_tensor("identb", [128, 128], bf16, kind="ExternalInput")
    idxua_d = nc.dram_tensor("idxua", [128, 3], mybir.dt.int32,
                             kind="ExternalInput")
    out_d = nc.dram_tensor("pred", [BL, NCH], f32, kind="ExternalOutput")
    if DEBUG:
        e_dbg = nc.dram_tensor("e_dbg", [NSTREAM, 128, 4, EMB], bf16,
                               kind="ExternalOutput")
        e0t_dbg = nc.dram_tensor("e0t_dbg", [BL, 128, 6, NMEM], bf16,
                                 kind="ExternalOutput")
        e1t_dbg = nc.dram_tensor("e1t_dbg", [BL, 128, 6, NMEM], bf16,
                                 kind="ExternalOutput")
        x_dbg = nc.dram_tensor("x_dbg", [128, EMB], f32,
                               kind="ExternalOutput")
        p_dbg = nc.dram_tensor("p_dbg", [128, NMEM], bf16,
                               kind="ExternalOutput")

    with tile.TileContext(nc) as tc:
        with (
            tc.tile_pool(name="const", bufs=1) as constp,
            tc.tile_pool(name="state", bufs=1) as statep,
            tc.tile_pool(name="selp1", bufs=24) as selp1,
            tc.tile_pool(name="selp2", bufs=4) as selp2,
            tc.tile_pool(name="wsp", bufs=2, space="PSUM") as wsp,
            tc.tile_pool(name="tpps", bufs=1, space="PSUM") as tpps,
            tc.tile_pool(name="yps", bufs=1, space="PSUM") as ypsp,
            tc.tile_pool(name="ops", bufs=1, space="PSUM") as opsp,
            tc.tile_pool(name="hps", bufs=1, space="PSUM") as hpsp,
        ):
            # ---- small consts first (gathers depend on idx/memid) ----
            idx_sb = [constp.tile([128, cmax], mybir.dt.int16, tag=f"idx{i}",
                                  name=f"idx{i}") for i in range(2)]
            mem_sb = [constp.tile([128, ngmax], f32, tag=f"mem{s}",
                                  name=f"mem{s}") for s in range(NSTREAM)]
            nc.sync.dma_start(out=idx_sb[0][:], in_=idx16_d[0])
            nc.sync.dma_start(out=mem_sb[0][:], in_=memid_d[0])
            iota = constp.tile([128, NMEM], mybir.dt.int16, tag="iota")
            nc.sync.dma_start(out=iota[:], in_=iota_d[:])
            identb = constp.tile([128, 128], bf16, tag="identb")
            nc.sync.dma_start(out=identb[:], in_=identb_d[:])

            # persistent state
            X = statep.tile([128, EMB], f32, tag="X")
            E = [statep.tile([128, 4, EMB], bf16, tag=f"E{s}", name=f"E{s}")
                 for s in range(NSTREAM)]
            E0T = [statep.tile([128, 6, NMEM], bf16, tag=f"E0T{b}",
                               name=f"E0T{b}") for b in range(BL)]
            E1T = [statep.tile([128, 6, NMEM], bf16, tag=f"E1T{b}",
                               name=f"E1T{b}") for b in range(BL)]
            G = [statep.tile([128, 4, EMB], bf16, tag=f"G{i}", name=f"G{i}")
                 for i in range(6)]
            S = hpsp.tile([128, NMEM], f32, tag="S")
            P = statep.tile([128, NMEM], bf16, tag="P")
            PT = statep.tile([128, 4, 128], bf16, tag="PT")
            o_sb = statep.tile([128, EMB], bf16, tag="o_sb")
            # renorm scalars [128,1]
            sc = {n: statep.tile([128, 1], f32, tag=n, name=n)
                  for n in ("mx", "mn", "sm", "negmx", "se", "lse", "ef",
                            "mf", "t1", "Av", "bq", "ba", "Bv", "invb")}

            # memset gather buffers once (NaN-safety for stale group tails)
            for i in range(6):
                nc.any.memset(G[i][:], 0.0)
            nc.vector.memset(S[:], 0.0)
            nc.any.memset(P[:], 0.0)

            # ---- remaining consts ----
            nc.sync.dma_start(out=idx_sb[1][:], in_=idx16_d[1])
            for s in range(1, NSTREAM):
                nc.sync.dma_start(out=mem_sb[s][:], in_=memid_d[s])
            identf = constp.tile([128, 128], f32, tag="identf")
            nc.sync.dma_start(out=identf[:], in_=identf_d[:])
            maskq = constp.tile([128, 1], f32, tag="maskq")
            nc.sync.dma_start(out=maskq[:], in_=maskq_d[:])
            ones3 = [constp.tile([128, 128], bf16, tag=f"ones{i}",
                                 name=f"ones{i}") for i in range(3)]
            for i in range(3):
                nc.sync.dma_start(out=ones3[i][:], in_=ones3_d[i])
            idxua = constp.tile([128, 3], mybir.dt.int32, tag="idxua")
            nc.sync.dma_start(out=idxua[:], in_=idxua_d[:])
            ut_sb = constp.tile([128, 6, EMB], bf16, tag="ut")
            vt_sb = constp.tile([128, 6, EMB], bf16, tag="vt")
            w_sb = constp.tile([128, 6, EMB], bf16, tag="w")
            for t_sb, t_d in ((ut_sb, ut_d), (vt_sb, vt_d), (w_sb, w_d)):
                nc.sync.dma_start(
                    out=t_sb[:],
                    in_=t_d[:].rearrange("(j p) d -> p j d", p=128))

            # ---- init: u and a from B_table -> X; preload Exp table ----
            # stage the 3 B-table gathers in G[0] (free until stream 0)
            for i in range(3):
                nc.gpsimd.indirect_dma_start(
                    out=G[0][:, i, :], out_offset=None, in_=b_tab[:],
                    in_offset=bass.IndirectOffsetOnAxis(
                        ap=idxua[:, i:i + 1], axis=0))
            ps0 = wsp.tile([128, EMB], f32, tag="wsp", name="ps0")
            for i in range(3):
                nc.tensor.matmul(out=ps0[:, 0:512], lhsT=ones3[i][:],
                                 rhs=G[0][:, i, 0:512],
                                 start=(i == 0), stop=(i == 2))
                nc.tensor.matmul(out=ps0[:, 512:768], lhsT=ones3[i][:],
                                 rhs=G[0][:, i, 512:768],
                                 start=(i == 0), stop=(i == 2))
            nc.vector.tensor_copy(out=X[:, 0:512], in_=ps0[:, 0:512])
            nc.vector.tensor_copy(out=X[:, 512:768], in_=ps0[:, 512:768])
            # warm the Exp activation table (only table ever used)
            nc.scalar.activation(out=sc["ef"][:], in_=maskq[:],
                                 func=mybir.ActivationFunctionType.Exp)

            # ---- helpers ----
            def xt_extract(src_sb, ident, xq, xa, dt):
                """Transpose [128, 768] src into per-b q/a column tiles."""
                for j in range(6):
                    tp = tpps.tile([128, 128], dt, tag="tp")
                    nc.tensor.transpose(
                        out=tp[:], in_=src_sb[:, 128 * j:128 * j + 128],
                        identity=ident[:])
                    v = tp[:].rearrange("p (b n) -> p b n", b=4)
                    nc.vector.tensor_copy(out=xq[:, j, :], in_=v[:, :, 0])
                    nc.vector.tensor_copy(out=xa[:, j, :], in_=v[:, :, 1:9])

            def y_project(xq, xa, ysb):
                for i in range(6):
                    y36 = ypsp.tile([128, 36], f32, tag="y36", name="y36")
                    for j in range(6):
                        nc.tensor.matmul(
                            out=y36[:, 0:4],
                            lhsT=ut_sb[:, j, 128 * i:128 * i + 128],
                            rhs=xq[:, j, :], start=(j == 0), stop=(j == 5))
                    for j in range(6):
                        nc.tensor.matmul(
                            out=y36[:, 4:36],
                            lhsT=vt_sb[:, j, 128 * i:128 * i + 128],
                            rhs=xa[:, j, :], start=(j == 0), stop=(j == 5))
                    yv = ysb[:, i, :].rearrange("p (b n) -> p b n", b=4)
                    nc.vector.tensor_copy(out=yv[:, :, 0], in_=y36[:, 0:4])
                    nc.vector.tensor_copy(out=yv[:, :, 1:9], in_=y36[:, 4:36])

            def renorm(rows, n):
                """Renorm S[rows:rows+n] -> P[rows:rows+n] (bf16)."""
                rs = slice(rows, rows + n)
                nc.vector.tensor_reduce(out=sc["mx"][rs], in_=S[rs, :],
                                        axis=mybir.AxisListType.X,
                                        op=mybir.AluOpType.max)
                nc.vector.tensor_scalar(out=sc["negmx"][rs], in0=sc["mx"][rs],
                                        scalar1=-1.0, scalar2=None,
                                        op0=mybir.AluOpType.mult)
                nc.scalar.activation(
                    out=P[rs, :], in_=S[rs, :],
                    func=mybir.ActivationFunctionType.Exp,
                    bias=sc["negmx"][rs], scale=1.0, accum_out=sc["se"][rs])
                nc.vector.tensor_reduce(out=sc["mn"][rs], in_=S[rs, :],
                                        axis=mybir.AxisListType.X,
                                        op=mybir.AluOpType.min)
                nc.vector.tensor_reduce(out=sc["sm"][rs], in_=S[rs, :],
                                        axis=mybir.AxisListType.X,
                                        op=mybir.AluOpType.add)
                # lse = mx + ln2 * log2(se); log2 via exponent + cubic poly
                se_i = sc["se"][rs].bitcast(mybir.dt.int32)
                ef_i = sc["ef"][rs].bitcast(mybir.dt.int32)
                nc.vector.tensor_scalar(out=ef_i, in0=se_i,
                                        scalar1=23, scalar2=None,
                                        op0=mybir.AluOpType.logical_shift_right)
                nc.vector.tensor_copy(out=sc["ef"][rs], in_=ef_i)
                mf_i = sc["mf"][rs].bitcast(mybir.dt.int32)
                nc.vector.tensor_scalar(out=mf_i, in0=se_i,
                                        scalar1=0x007FFFFF,
                                        scalar2=0x3F800000,
                                        op0=mybir.AluOpType.bitwise_and,
                                        op1=mybir.AluOpType.bitwise_or)
                m = sc["mf"][rs]
                t1 = sc["t1"][rs]
                nc.vector.tensor_scalar(out=t1, in0=m, scalar1=C3, scalar2=C2,
                                        op0=mybir.AluOpType.mult,
                                        op1=mybir.AluOpType.add)
                nc.vector.tensor_tensor(out=t1, in0=t1, in1=m,
                                        op=mybir.AluOpType.mult)
                nc.vector.tensor_scalar(out=t1, in0=t1, scalar1=C1,
                                        scalar2=None,
                                        op0=mybir.AluOpType.add)
                nc.vector.tensor_tensor(out=t1, in0=t1, in1=m,
                                        op=mybir.AluOpType.mult)
                nc.vector.tensor_scalar(out=t1, in0=t1, scalar1=C0 - 127.0,
                                        scalar2=None,
                                        op0=mybir.AluOpType.add)
                nc.vector.tensor_tensor(out=t1, in0=t1, in1=sc["ef"][rs],
                                        op=mybir.AluOpType.add)
                nc.vector.tensor_scalar(out=sc["lse"][rs], in0=t1,
                                        scalar1=LN2, scalar2=sc["mx"][rs],
                                        op0=mybir.AluOpType.mult,
                                        op1=mybir.AluOpType.add)
                # A = lse + maskq*(mn - lse)
                nc.vector.tensor_tensor(out=t1, in0=sc["mn"][rs],
                                        in1=sc["lse"][rs],
                                        op=mybir.AluOpType.subtract)
                nc.vector.tensor_tensor(out=t1, in0=t1, in1=maskq[rs],
                                        op=mybir.AluOpType.mult)
                nc.vector.tensor_tensor(out=sc["Av"][rs], in0=sc["lse"][rs],
                                        in1=t1, op=mybir.AluOpType.add)
                # Bq = sm - 512*mn + 512e-8 ; Ba = 512*lse - sm
                nc.vector.tensor_scalar(out=sc["bq"][rs], in0=sc["mn"][rs],
                                        scalar1=-512.0, scalar2=512e-8,
                                        op0=mybir.AluOpType.mult,
                                        op1=mybir.AluOpType.add)
                nc.vector.tensor_tensor(out=sc["bq"][rs], in0=sc["bq"][rs],
                                        in1=sc["sm"][rs],
                                        op=mybir.AluOpType.add)
                nc.vector.tensor_scalar(out=sc["ba"][rs], in0=sc["lse"][rs],
                                        scalar1=512.0, scalar2=None,
                                        op0=mybir.AluOpType.mult)
                nc.vector.tensor_tensor(out=sc["ba"][rs], in0=sc["ba"][rs],
                                        in1=sc["sm"][rs],
                                        op=mybir.AluOpType.subtract)
                nc.vector.tensor_tensor(out=sc["Bv"][rs], in0=sc["bq"][rs],
                                        in1=sc["ba"][rs],
                                        op=mybir.AluOpType.subtract)
                nc.vector.tensor_tensor(out=sc["Bv"][rs], in0=sc["Bv"][rs],
                                        in1=maskq[rs],
                                        op=mybir.AluOpType.mult)
                nc.vector.tensor_tensor(out=sc["Bv"][rs], in0=sc["Bv"][rs],
                                        in1=sc["ba"][rs],
                                        op=mybir.AluOpType.add)
                nc.vector.reciprocal(out=sc["invb"][rs], in_=sc["Bv"][rs])
                for kk in range(4):
                    nc.vector.tensor_scalar(
                        out=P[rs, 128 * kk:128 * kk + 128],
                        in0=S[rs, 128 * kk:128 * kk + 128],
                        scalar1=sc["Av"][rs],
                        scalar2=sc["invb"][rs],
                        op0=mybir.AluOpType.subtract,
                        op1=mybir.AluOpType.mult)

            # hop-0 Y from initial X (runs during early gathers)
            xtq0 = statep.tile([128, 6, 4], bf16, tag="xtq0")
            xta0 = statep.tile([128, 6, 32], bf16, tag="xta0")
            ysb0 = statep.tile([128, 6, 36], bf16, tag="ysb0")
            xt_extract(X, identf, xtq0, xta0, f32)
            y_project(xtq0, xta0, ysb0)

            # ---- gather streams + word-sum + inline hop 0 ----
            piece_ctr = 0
            pending = []

            def act_copy(out, in_):
                nc.scalar.activation(
                    out=out, in_=in_,
                    func=mybir.ActivationFunctionType.Copy)
            for s in range(NSTREAM):
                b, tau = divmod(s, 3)
                lo_c, hi_c = LO[s], HI[s]
                nglo = -(-lo_c // 128)
                ng = nglo + (-(-hi_c // 128))
                runs = _runs(gmap[s], nglo, ng)
                # chunk psum tiles keyed (c, phase)
                pst = {}
                done_groups = {}

                if s + 1 < NSTREAM:
                    nc.sync.dma_start(out=idx_sb[(s + 1) % 2][:],
                                      in_=idx16_d[s + 1])
                sections = [(0, lo_c, 0, a_cat[:, :]),
                            (nglo, hi_c, lo_c, a_cat[SPLIT:, :])]
                for gbase, size, stream_off, src in sections:
                    for off, n in _pieces(size):
                        gt = G[piece_ctr % 6]
                        piece_ctr += 1
                        npg = -(-n // 128)
                        nc.gpsimd.dma_gather(
                            gt[:, 0:npg, :], src,
                            idx_sb[s % 2][:, (stream_off + off) // 16:
                                          (stream_off + off + n) // 16],
                            n, n, EMB)
                        if pending:
                            pending.pop(0)()
                        g0 = gbase + off // 128
                        sels = {}
                        for gl in range(npg):
                            g = g0 + gl
                            cs = gmap[s][g]
                            if not cs:
                                continue
                            c0, c1 = cs[0], cs[-1]
                            w = 128 * (c1 - c0 + 1)
                            if w > 128:
                                sel = selp2.tile([128, 256], bf16, tag="sl2",
                                                 name="sl2")
                            else:
                                sel = selp1.tile([128, 128], bf16, tag="sl1",
                                                 name="sl1")
                            nc.vector.tensor_scalar(
                                out=sel[:, 0:w],
                                in0=iota[:, 128 * c0:128 * c0 + w],
                                scalar1=mem_sb[s][:, g:g + 1], scalar2=None,
                                op0=mybir.AluOpType.is_equal)
                            sels[gl] = (sel, c0)
                        for gl in range(npg):
                            g = g0 + gl
                            cs = gmap[s][g]
                            if not cs:
                                continue
                            sel, c0 = sels[gl]
                            ph = 0 if g < nglo else 1
                            for c in cs:
                                k = (c, ph)
                                if k not in pst:
                                    pst[k] = wsp.tile([128, EMB], f32,
                                                      tag="wsp", name="pw")
                                pw = pst[k]
                                pa, pb = pw[:, 0:512], pw[:, 512:768]
                                lhs = sel[:, 128 * (c - c0):
                                          128 * (c - c0) + 128]
                                first = runs[k][0] == g
                                last = runs[k][1] == g
                                nc.tensor.matmul(
                                    out=pa, lhsT=lhs, rhs=gt[:, gl, 0:512],
                                    start=first, stop=last)
                                nc.tensor.matmul(
                                    out=pb, lhsT=lhs,
                                    rhs=gt[:, gl, 512:768],
                                    start=first, stop=last)
                                if last:
                                    done_groups[k] = True
                                    pst.pop(k)
                                    if ph == 0:
                                        act_copy(E[s][:, c, 0:512], pa)
                                        act_copy(E[s][:, c, 512:768], pb)
                                    else:
                                        nc.vector.tensor_tensor(
                                            out=E[s][:, c, 0:512],
                                            in0=E[s][:, c, 0:512], in1=pa,
                                            op=mybir.AluOpType.add)
                                        nc.vector.tensor_tensor(
                                            out=E[s][:, c, 512:768],
                                            in0=E[s][:, c, 512:768],
                                            in1=pb,
                                            op=mybir.AluOpType.add)
                                        # inline transposes for E0T / E1T
                                        for q in range(2):
                                            tp = tpps.tile([128, 128], bf16,
                                                           tag="tp")
                                            nc.tensor.transpose(
                                                out=tp[:],
                                                in_=E[s][:, c, 128 * q:
                                                         128 * q + 128],
                                                identity=identb[:])
                                            nc.vector.tensor_copy(
                                                out=E0T[b][:, 2 * tau + q,
                                                           128 * c:
                                                           128 * c + 128],
                                                in_=tp[:])
                                        for q in range(2):
                                            tp = tpps.tile([128, 128], bf16,
                                                           tag="tp")
                                            nc.tensor.transpose(
                                                out=tp[:],
                                                in_=E[s][:, c, 256 + 128 * q:
                                                         256 + 128 * q + 128],
                                                identity=identb[:])
                                            act_copy(
                                                E1T[b][:, 2 * tau + q,
                                                       128 * c:
                                                       128 * c + 128],
                                                tp[:])

                # ---- inline hop 0 for batch b once its 3 streams done ----
                if tau == 2:
                    def hop0_scores(b=b):
                        r0 = 32 * b
                        for j in range(6):
                            nc.tensor.matmul(
                                out=S[r0:r0 + 9, :],
                                lhsT=ysb0[:, j, 9 * b:9 * b + 9],
                                rhs=E0T[b][:, j, :], start=(j == 0),
                                stop=(j == 5), tile_position=(0, r0))

                    def hop0_renorm(b=b):
                        renorm(32 * b, 9)

                    def hop0_pt(b=b):
                        r0 = 32 * b
                        for k in range(4):
                            tp = tpps.tile([128, 128], bf16, tag="tp")
                            nc.tensor.transpose(
                                out=tp[:],
                                in_=P[:, 128 * k:128 * k + 128],
                                identity=identb[:])
                            nc.vector.tensor_copy(out=PT[:, k, r0:r0 + 9],
                                                  in_=tp[:, r0:r0 + 9])

                    def hop0_o(b=b, t2=0):
                        r0 = 32 * b
                        if t2 == 0:
                            odst = S[r0:r0 + 9, 0:256]
                        elif t2 == 1:
                            odst = S[r0:r0 + 9, 256:512]
                        else:
                            ot = opsp.tile([128, 256], f32, tag="ot",
                                           name="ot")
                            hop0_o.ot = ot
                            odst = ot[r0:r0 + 9, :]
                        for k in range(4):
                            nc.tensor.matmul(
                                out=odst,
                                lhsT=PT[:, k, r0:r0 + 9],
                                rhs=E[3 * b + t2][:, k, 256:512],
                                start=(k == 0), stop=(k == 3),
                                tile_position=(0, r0))
                        nc.vector.tensor_tensor(
                            out=X[r0:r0 + 9, 256 * t2:256 * t2 + 256],
                            in0=X[r0:r0 + 9, 256 * t2:256 * t2 + 256],
                            in1=odst,
                            op=mybir.AluOpType.add)

                    if b < BL - 1:
                        pending.extend([
                            hop0_scores, hop0_renorm, hop0_pt,
                            lambda b=b: hop0_o(b, 0),
                            lambda b=b: hop0_o(b, 1),
                            lambda b=b: hop0_o(b, 2)])
                    else:
                        hop0_scores()
                        hop0_renorm()
                        hop0_pt()
                        for t2 in range(3):
                            hop0_o(b, t2)

            while pending:
                pending.pop(0)()

            if DEBUG:
                for s in range(NSTREAM):
                    nc.sync.dma_start(out=e_dbg[s], in_=E[s][:])
                for b in range(BL):
                    nc.sync.dma_start(out=e0t_dbg[b], in_=E0T[b][:])
                    nc.sync.dma_start(out=e1t_dbg[b], in_=E1T[b][:])
                nc.sync.dma_start(out=x_dbg[:], in_=X[:])
                nc.sync.dma_start(out=p_dbg[:], in_=P[:])

            # ---------------- hop 1 + final ----------------
            xtq1 = statep.tile([128, 6, 4], bf16, tag="xtq1")
            xta1 = statep.tile([128, 6, 32], bf16, tag="xta1")
            ysb1 = statep.tile([128, 6, 36], bf16, tag="ysb1")
            xt_extract(X, identf, xtq1, xta1, f32)
            y_project(xtq1, xta1, ysb1)
            for b in range(BL):
                r0 = 32 * b
                for j in range(6):
                    nc.tensor.matmul(
                        out=S[r0:r0 + 9, :], lhsT=ysb1[:, j, 9 * b:9 * b + 9],
                        rhs=E1T[b][:, j, :], start=(j == 0), stop=(j == 5),
                        tile_position=(0, r0))
            renorm(0, 128)
            for k in range(4):
                tp = tpps.tile([128, 128], bf16, tag="tp")
                nc.tensor.transpose(out=tp[:], in_=P[:, 128 * k:128 * k + 128],
                                    identity=identb[:])
                nc.vector.tensor_copy(out=PT[:, k, :], in_=tp[:])
            ot1 = opsp.tile([128, 256], f32, tag="ot", name="ot1")
            nc.vector.memset(ot1[:], 0.0)
            for b in range(BL):
                r0 = 32 * b
                odst = [S[r0:r0 + 9, 0:256], S[r0:r0 + 9, 256:512],
                        ot1[r0:r0 + 9, :]]
                for t2 in range(3):
                    for k in range(4):
                        nc.tensor.matmul(
                            out=odst[t2],
                            lhsT=PT[:, k, r0:r0 + 9],
                            rhs=E[3 * b + t2][:, k, 512:768],
                            start=(k == 0), stop=(k == 3),
                            tile_position=(0, r0))
            nc.vector.tensor_copy(out=o_sb[:, 0:256], in_=S[:, 0:256])
            nc.vector.tensor_copy(out=o_sb[:, 256:512], in_=S[:, 256:512])
            nc.vector.tensor_copy(out=o_sb[:, 512:768], in_=ot1[:])

            # final bilinear form: pred[b,c] = o_q[b] . (W @ o_a[c,b])
            otq = statep.tile([128, 6, 4], bf16, tag="otq")
            ota = statep.tile([128, 6, 32], bf16, tag="ota")
            xt_extract(o_sb, identb, otq, ota, bf16)
            wq = statep.tile([128, 6, 4], bf16, tag="wq")
            for i in range(6):
                wqp = ypsp.tile([128, 36], f32, tag="y36", name="wqp")
                for j in range(6):
                    nc.tensor.matmul(
                        out=wqp[:, 0:4],
                        lhsT=w_sb[:, j, 128 * i:128 * i + 128],
                        rhs=otq[:, j, :], start=(j == 0), stop=(j == 5))
                nc.vector.tensor_copy(out=wq[:, i, :], in_=wqp[:, 0:4])
            predp36 = ypsp.tile([128, 36], f32, tag="y36", name="predp36")
            predp = predp36[:, 0:NCH]
            pred_sb = statep.tile([128, NCH], f32, tag="pred_sb")
            for b in range(BL):
                for i in range(6):
                    nc.tensor.matmul(
                        out=predp36[32 * b:32 * b + 1, 0:NCH],
                        lhsT=wq[:, i, b:b + 1],
                        rhs=ota[:, i, 8 * b:8 * b + 8],
                        start=(i == 0), stop=(i == 5),
                        tile_position=(0, 32 * b))
                nc.vector.tensor_copy(out=pred_sb[32 * b:32 * b + 1, :],
                                      in_=predp36[32 * b:32 * b + 1, 0:NCH])
                nc.sync.dma_start(out=out_d[b:b + 1, :],
                                  in_=pred_sb[32 * b:32 * b + 1, :])

    nc.compile()
    _cache[key] = nc
    return nc


def prepare(subjects, relations, objects, ques, answerChoices,
            A_tables, B_table, U, V, W):
    subjects = np.asarray(subjects).astype(np.int64)
    relations = np.asarray(relations).astype(np.int64)
    objects = np.asarray(objects).astype(np.int64)
    ques = np.asarray(ques).astype(np.int64)
    answerChoices = np.asarray(answerChoices).astype(np.int64)
    A_tables = np.asarray(A_tables, dtype=np.float32)
    B_table = np.asarray(B_table, dtype=np.float32)

    # shared (core-independent) device data
    a_cat = np.concatenate([A_tables[0], A_tables[1], A_tables[2]],
                           axis=1).astype(BF)
    b_bf = B_table.astype(BF)
    ut = np.ascontiguousarray(np.asarray(U, dtype=np.float32).T).astype(BF)
    vt = np.ascontiguousarray(np.asarray(V, dtype=np.float32).T).astype(BF)
    w_bf = np.ascontiguousarray(np.asarray(W, dtype=np.float32)).astype(BF)
    identf = np.eye(128, dtype=np.float32)
    identb = np.eye(128, dtype=BF)
    maskq = np.zeros((128, 1), dtype=np.float32)
    maskq[0::32] = 1.0
    iota = np.tile(np.arange(NMEM, dtype=np.int16), (128, 1))
    # init placement matrices (state row = 32*b + 0 for u, +1+c for choices)
    ones3 = np.zeros((3, 128, 128), dtype=BF)
    p = np.arange(128)
    ones3[0, p, 32 * (p // 32)] = 1.0                        # u rows
    ones3[1, p, 32 * (p // 64) + 1 + (p // 8) % 8] = 1.0     # a, b in {0,1}
    ones3[2, p, 32 * (2 + p // 64) + 1 + (p // 8) % 8] = 1.0  # a, b in {2,3}

    LO, HI, gmap = _build_structure(subjects, relations, objects)
    cmax = max((LO[s] + HI[s]) // 16 for s in range(NSTREAM))
    ngmax = max(-(-LO[s] // 128) - (-HI[s] // 128) for s in range(NSTREAM))
    nc = _build_program(LO, HI, gmap, cmax, ngmax)

    in_maps = []
    for core in range(NCORES):
        sl = slice(core * BL, (core + 1) * BL)
        idx16, memid, _, _ = _build_core_data(
            subjects[sl], relations[sl], objects[sl], LO, HI)
        idxua = np.zeros((128, 3), dtype=np.int32)
        idxua[:, 0] = ques[sl][p // 32, p % 32]
        idxua[:, 1] = answerChoices[sl][p // 64, (p // 8) % 8, p % 8]
        idxua[:, 2] = answerChoices[sl][2 + p // 64, (p // 8) % 8, p % 8]
        in_maps.append(dict(
            a_cat=a_cat, b_tab=b_bf, ut=ut, vt=vt, w=w_bf,
            idx16=idx16, memid=memid, iota=iota, ones3=ones3, maskq=maskq,
            identf=identf, identb=identb, idxua=idxua))

    return nc, in_maps


def kernel(**inputs):
    nc, in_maps = prepare(**inputs)
    res = run_bass_kernel_spmd(nc, in_maps, list(range(NCORES)), trace=TRACE)
    global LAST_RESULTS
    LAST_RESULTS = res
    return np.concatenate([res.results[c]["pred"] for c in range(NCORES)],
                          axis=0).astype(np.float32)
